# revision 8
# baseline (speedup 1.0000x reference)
"""Multi-head attention (B=2, S=2048, D=1024, H=16, causal mask) on 8 TRN2 cores.

Sharding: core c handles batch b = c // 4 and head-group hg = c % 4
(4 heads = 256 feature dims each). Each core computes its heads' QKV
projections, causal attention, and a partial output projection
(attn_out @ w_o[:, hg].T); the host sums the 4 partials per batch and
adds b_o.

Device layout (all chosen to avoid on-chip transposes):
  - host passes x.T [D, S] so projections contract d on partitions
  - Q,K kept transposed [dk, s]; V kept natural [s, dv] with a ones
    column appended -> PV matmul also emits softmax row sums
  - scores computed transposed S_T[k, q]; softmax without max
    subtraction (scores are O(+-8) for this data distribution)

Perf-critical structure (v2), from trace analysis of v1:
  - the PE runs at 2.4 GHz only after ~3-16us of continuous execution
    and drops to 1.2 GHz when the stream has gaps, so emission order
    software-pipelines heads (scores h+1 before PV h, broadcast 2
    steps late, out-projection of qc interleaved into qc+1) to keep
    the tensor stream dependency-free
  - exp() reads score PSUM directly (removes the 54us DVE psum->sbuf
    copy stream of v1)
  - causal diagonal masking: a constant 128x128 triangular matrix is
    matmul-preloaded into PSUM (start=True), the score matmul then
    accumulates on top (start=False) - no DVE staircase adds
  - softmax normalize: DVE reciprocal on the row-sum line, broadcast
    64-wide via a float32r rank-1 matmul (1 cycle/col vs 4 for fp32),
    then one DVE multiply into the attention-out accumulator
  - output in bf16 (halves write traffic; host sums partials in fp32)
  - all HBM traffic as exactly 16 large HWDGE DMAs (8 queues x 2 uses,
    no ring-credit waits, <=1 sync wait per DMA after _split_multi_waits)
"""

import sys

if "/opt/trn_rl_repo" not in sys.path:
    sys.path.insert(0, "/opt/trn_rl_repo")

import numpy as np
import ml_dtypes

BF16 = ml_dtypes.bfloat16

B, S, D, H = 2, 2048, 1024, 16
NCORE = 8
HGROUPS = 4  # head-groups == cores per batch
HPC = H // HGROUPS  # heads per core = 4
DK = D // H  # head dim = 64
DKB = HPC * DK  # feature dims per core = 256
P = 128
QC = 512  # q chunk (one PSUM bank of fp32)
NEG = -1e9
MASKVAL = -30000.0  # exp(0.125*(s+MASKVAL)) == 0 for any realistic s

_nc_cache = {}


def _build_causal(seq=S):
    import concourse.bass as bass
    import concourse.tile as tile
    from concourse import mybir
    from contextlib import ExitStack

    f32 = mybir.dt.float32
    f32r = mybir.dt.float32r
    bf16 = mybir.dt.bfloat16
    exp_fn = mybir.ActivationFunctionType.Exp
    nqc = seq // QC
    nkt = seq // P
    nd = D // P  # 8 d-chunks
    WREST = 3 * 2048 + 2 * P  # wk, wv, wo, tri, ident

    nc = bass.Bass()
    xq_d = nc.dram_tensor("xq_t", [D, seq], bf16, kind="ExternalInput")
    xk_d = nc.dram_tensor("xk_t", [D, seq], bf16, kind="ExternalInput")
    xv_d = nc.dram_tensor("xv_t", [D, seq], bf16, kind="ExternalInput")
    wq_d = nc.dram_tensor("wq_p", [P, D * DKB // P], bf16, kind="ExternalInput")
    wrest_d = nc.dram_tensor("wrest", [P, WREST], bf16, kind="ExternalInput")
    out_d = nc.dram_tensor("out", [seq, D], bf16, kind="ExternalOutput")

    with ExitStack() as ctx:
        tc = ctx.enter_context(tile.TileContext(nc))
        persist = ctx.enter_context(tc.tile_pool(name="persist", bufs=1))

        wrest_t = persist.tile([P, WREST], bf16, tag="wrest")

        QT, KT, AT, vt = [], [], [], []
        for m in range(2):
            QT.append(persist.tile([P, seq], bf16, tag=f"qt{m}", name=f"qt{m}"))
            KT.append(persist.tile([P, seq], bf16, tag=f"kt{m}", name=f"kt{m}"))
            AT.append(persist.tile([P, seq], bf16, tag=f"at{m}", name=f"at{m}"))

        # ---- phase 1: projections (own PSUM + x pools, released after) ----
        with tc.tile_pool(name="xpool", bufs=1) as xpool, tc.tile_pool(
            name="projp", bufs=2, space="PSUM"
        ) as projp:
            wq_t = xpool.tile([P, D * DKB // P], bf16, tag="wq")
            # DMA issue order fills the 8 HWDGE queues round-robin; the
            # second wave (8 output DMAs) reuses them once each.
            nc.sync.dma_start(out=wq_t[:], in_=wq_d[:, :])

            def load_xt(xdram, name):
                t = xpool.tile([P, nd, seq], bf16, tag=name, name=name)
                h = seq // 2
                nc.sync.dma_start(
                    out=t[:, :, 0:h],
                    in_=xdram[:, 0:h].rearrange("(j p) s -> p j s", p=P),
                )
                nc.sync.dma_start(
                    out=t[:, :, h:seq],
                    in_=xdram[:, h:seq].rearrange("(j p) s -> p j s", p=P),
                )
                return t

            xq_t = load_xt(xq_d, "xq")
            xk_t = load_xt(xk_d, "xk")
            xv_t = load_xt(xv_d, "xv")
            nc.sync.dma_start(out=wrest_t[:], in_=wrest_d[:, :])

            def project_T(xt, wtile, res, name):
                ngroups = [
                    list(range(i, min(i + 2, nqc))) for i in range(0, nqc, 2)
                ]
                for m in range(2):
                    for gi, grp in enumerate(ngroups):
                        ps = projp.tile(
                            [P, len(grp) * QC],
                            f32,
                            tag="pj",
                            name=f"ps_{name}{m}_{gi}",
                        )
                        for half, n in enumerate(grp):
                            for j in range(nd):
                                nc.tensor.matmul(
                                    ps[:, half * QC : (half + 1) * QC],
                                    lhsT=wtile[
                                        :, j * DKB + m * P : j * DKB + (m + 1) * P
                                    ],
                                    rhs=xt[:, j, n * QC : (n + 1) * QC],
                                    start=(j == 0),
                                    stop=(j == nd - 1),
                                )
                        nc.vector.tensor_copy(
                            out=res[m][:, grp[0] * QC : (grp[-1] + 1) * QC],
                            in_=ps[:],
                        )

            wk_t = wrest_t[:, 0:2048]
            wv_t = wrest_t[:, 2048:4096]
            wo_t = wrest_t[:, 4096:6144]
            tri_t = wrest_t[:, 6144 : 6144 + P]
            idn_t = wrest_t[:, 6144 + P : 6144 + 2 * P]

            project_T(xq_t, wq_t, QT, "qt")
            project_T(xk_t, wk_t, KT, "kt")

            # V natural layout [s, dv] + ones column per head
            for st in range(nkt):
                ps = projp.tile([P, DKB], f32, tag="pj", name=f"ps_v{st}")
                for j in range(nd):
                    nc.tensor.matmul(
                        ps[:],
                        lhsT=xv_t[:, j, st * P : (st + 1) * P],
                        rhs=wv_t[:, j * DKB : (j + 1) * DKB],
                        start=(j == 0),
                        stop=(j == nd - 1),
                    )
                v = persist.tile(
                    [P, HPC * (DK + 1)], bf16, tag=f"v{st}", name=f"v{st}"
                )
                nc.vector.memset(v[:], 1.0)
                nc.vector.tensor_copy(
                    out=v[:].rearrange("p (h w) -> p h w", w=DK + 1)[:, :, 0:DK],
                    in_=ps[:].rearrange("p (h w) -> p h w", w=DK),
                )
                vt.append(v)

        # ---- phase 2: attention + interleaved output projection ----
        # PSUM: st 2x2 banks + pv 2x1 + fp (bcast/outproj) 2x1 = 8 banks
        st_ps = ctx.enter_context(tc.tile_pool(name="st_ps", bufs=2, space="PSUM"))
        pv_ps = ctx.enter_context(tc.tile_pool(name="pv_ps", bufs=2, space="PSUM"))
        fp_ps = ctx.enter_context(tc.tile_pool(name="fp_ps", bufs=2, space="PSUM"))
        attn_pool = ctx.enter_context(tc.tile_pool(name="attn_pool", bufs=16))
        inv_pool = ctx.enter_context(tc.tile_pool(name="inv_pool", bufs=4))
        binv_pool = ctx.enter_context(tc.tile_pool(name="binv_pool", bufs=2))
        outp = ctx.enter_context(tc.tile_pool(name="outp", bufs=2))

        def trim(kt, qc):
            o = kt * P - qc * QC
            return (o, True) if o >= 0 else (0, False)

        def emit_scores(qc, h):
            """Score matmuls + exp for all kt-pairs of (qc, h). Diagonal
            tiles get the triangular mask matmul-preloaded into PSUM."""
            hm, hp = divmod(h, 2)
            hp *= DK
            kts = list(range(4 * (qc + 1)))
            pairs = [kts[i : i + 2] for i in range(0, len(kts), 2)]
            ats = []
            for pi, pair in enumerate(pairs):
                stt = st_ps.tile(
                    [P, 2 * QC], f32, tag="st", name=f"st{qc}_{h}_{pi}"
                )
                for half, kt in enumerate(pair):
                    o, diag = trim(kt, qc)
                    b = half * QC
                    kl = KT[hm][hp : hp + DK, kt * P : (kt + 1) * P]
                    if diag:
                        nc.tensor.matmul(
                            stt[:, b + o : b + o + P],
                            lhsT=tri_t,
                            rhs=idn_t,
                            start=True,
                            stop=False,
                            skip_group_check=True,
                        )
                        nc.tensor.matmul(
                            stt[:, b + o : b + o + P],
                            lhsT=kl,
                            rhs=QT[hm][
                                hp : hp + DK, qc * QC + o : qc * QC + o + P
                            ],
                            start=False,
                            stop=True,
                            skip_group_check=True,
                        )
                        if o + P < QC:
                            nc.tensor.matmul(
                                stt[:, b + o + P : b + QC],
                                lhsT=kl,
                                rhs=QT[hm][
                                    hp : hp + DK,
                                    qc * QC + o + P : (qc + 1) * QC,
                                ],
                                start=True,
                                stop=True,
                                skip_group_check=True,
                            )
                    else:
                        nc.tensor.matmul(
                            stt[:, b : b + QC],
                            lhsT=kl,
                            rhs=QT[hm][hp : hp + DK, qc * QC : (qc + 1) * QC],
                            start=True,
                            stop=True,
                        )
                at = attn_pool.tile(
                    [P, 2 * QC], bf16, tag="attn", name=f"a{qc}_{h}_{pi}"
                )
                o0, d0 = trim(pair[0], qc)
                o1 = trim(pair[1], qc)[0]
                if d0:
                    spans = [(o0, QC), (QC + o1, 2 * QC)]
                else:
                    spans = [(0, 2 * QC)]
                for lo, hi in spans:
                    nc.scalar.activation(
                        out=at[:, lo:hi],
                        in_=stt[:, lo:hi],
                        func=exp_fn,
                        scale=0.125,
                    )
                ats.append((at, pair))
            return ats

        def emit_pv(qc, h, ats):
            pv = pv_ps.tile([DK + 1, QC], f32, tag="pv", name=f"pv{qc}_{h}")
            last = 4 * (qc + 1) - 1
            for at, pair in ats:
                for half, kt in enumerate(pair):
                    o, _ = trim(kt, qc)
                    nc.tensor.matmul(
                        pv[:, o:QC],
                        lhsT=vt[kt][:, h * (DK + 1) : (h + 1) * (DK + 1)],
                        rhs=at[:, half * QC + o : (half + 1) * QC],
                        start=(kt == 0),
                        stop=(kt == last),
                        skip_group_check=True,
                    )
            inv = inv_pool.tile([1, QC], bf16, tag="inv", name=f"inv{qc}_{h}")
            with nc.allow_low_precision(reason="bf16 1/sums broadcast"):
                nc.vector.reciprocal(out=inv[:], in_=pv[DK : DK + 1, :])
            return pv, inv

        ones64 = persist.tile([1, DK], bf16, tag="ones64")
        nc.vector.memset(ones64[:], 1.0)

        def emit_bcast_mul(qc, h, pv, inv):
            # rank-1 bf16 matmul broadcasts 1/sums across 64 partitions
            # (1 cycle/col vs 4 for fp32); DVE stages it to SBUF so the
            # final multiply has only one PSUM operand (pv).
            hm, hp = divmod(h, 2)
            hp *= DK
            bc = fp_ps.tile([DK, QC], f32, tag="fp", name=f"bc{qc}_{h}")
            nc.tensor.matmul(
                bc[:], lhsT=ones64[:], rhs=inv[:], start=True, stop=True
            )
            binv = binv_pool.tile([DK, QC], f32, tag="binv", name=f"bi{qc}_{h}")
            nc.vector.tensor_copy(out=binv[:], in_=bc[:])
            nc.vector.tensor_mul(
                AT[hm][hp : hp + DK, qc * QC : (qc + 1) * QC],
                pv[0:DK, :],
                binv[:],
            )

        def emit_outproj(qc):
            for j2 in range(qc * (QC // (2 * P)), (qc + 1) * (QC // (2 * P))):
                ob = outp.tile([P, 2, D], bf16, tag="ob", name=f"ob{j2}")
                for g in range(2):
                    st = 2 * j2 + g
                    for nch in range(D // QC):
                        ps = fp_ps.tile(
                            [P, QC], f32, tag="fp", name=f"ps_o{st}_{nch}"
                        )
                        for m in range(2):
                            nc.tensor.matmul(
                                ps[:],
                                lhsT=AT[m][:, st * P : (st + 1) * P],
                                rhs=wo_t[
                                    :, m * D + nch * QC : m * D + (nch + 1) * QC
                                ],
                                start=(m == 0),
                                stop=(m == 1),
                            )
                        nc.vector.tensor_copy(
                            out=ob[:, g, nch * QC : (nch + 1) * QC], in_=ps[:]
                        )
                nc.sync.dma_start(
                    out=out_d[j2 * 2 * P : (j2 + 1) * 2 * P, :].rearrange(
                        "(g p) n -> p g n", p=P
                    ),
                    in_=ob[:],
                )

        # software-pipelined emission: broadcasts run 2 steps late and the
        # out-projection one qc late so the PE never waits on DVE/scalar
        pend_norm = []
        pend_out = None
        for qc in range(nqc):
            ats0 = emit_scores(qc, 0)
            ats1 = emit_scores(qc, 1)
            while pend_norm:
                emit_bcast_mul(*pend_norm.pop(0))
            pv0, inv0 = emit_pv(qc, 0, ats0)
            if pend_out is not None:
                emit_outproj(pend_out)
            ats2 = emit_scores(qc, 2)
            pv1, inv1 = emit_pv(qc, 1, ats1)
            emit_bcast_mul(qc, 0, pv0, inv0)
            ats3 = emit_scores(qc, 3)
            pv2, inv2 = emit_pv(qc, 2, ats2)
            emit_bcast_mul(qc, 1, pv1, inv1)
            pv3, inv3 = emit_pv(qc, 3, ats3)
            emit_bcast_mul(qc, 2, pv2, inv2)
            pend_norm = [(qc, 3, pv3, inv3)]
            pend_out = qc
        while pend_norm:
            emit_bcast_mul(*pend_norm.pop(0))
        emit_outproj(pend_out)

    return nc


def _build_legacy(mask_mode, seq=S):
    """Fallback for non-causal masks (mask_mode: 'none'|'full')."""
    import concourse.bass as bass
    import concourse.tile as tile
    from concourse import mybir
    from contextlib import ExitStack

    f32 = mybir.dt.float32
    bf16 = mybir.dt.bfloat16
    nqc = seq // QC
    nkt = seq // P
    nd = D // P  # 8 d-chunks

    nc = bass.Bass(num_swdge_queues=4)
    xq_d = nc.dram_tensor("xq_t", [D, seq], bf16, kind="ExternalInput")
    xk_d = nc.dram_tensor("xk_t", [D, seq], bf16, kind="ExternalInput")
    xv_d = nc.dram_tensor("xv_t", [D, seq], bf16, kind="ExternalInput")
    wq_d = nc.dram_tensor("wq_p", [P, D * DKB // P], bf16, kind="ExternalInput")
    wk_d = nc.dram_tensor("wk_p", [P, D * DKB // P], bf16, kind="ExternalInput")
    wv_d = nc.dram_tensor("wv_p", [P, D * DKB // P], bf16, kind="ExternalInput")
    wo_d = nc.dram_tensor("wo_p", [P, DKB * D // P], bf16, kind="ExternalInput")
    if mask_mode == "full":
        maskt_d = nc.dram_tensor("mask_t", [seq, seq], bf16, kind="ExternalInput")
    out_d = nc.dram_tensor("out", [seq, D], f32, kind="ExternalOutput")

    with ExitStack() as ctx:
        tc = ctx.enter_context(tile.TileContext(nc))
        persist = ctx.enter_context(tc.tile_pool(name="persist", bufs=1))

        ones64 = persist.tile([1, DK], f32, tag="ones64")
        nc.vector.memset(ones64[:], 1.0)
        wq_t = persist.tile([P, D * DKB // P], bf16, tag="wq")
        wk_t = persist.tile([P, D * DKB // P], bf16, tag="wk")
        wv_t = persist.tile([P, D * DKB // P], bf16, tag="wv")
        wo_t = persist.tile([P, DKB * D // P], bf16, tag="wo")
        nc.gpsimd.dma_start(out=wq_t[:], in_=wq_d[:, :])
        nc.gpsimd.dma_start(out=wk_t[:], in_=wk_d[:, :])
        nc.gpsimd.dma_start(out=wv_t[:], in_=wv_d[:, :])
        nc.gpsimd.dma_start(out=wo_t[:], in_=wo_d[:, :])

        QT, KT, vt = [], [], []
        for m in range(2):
            QT.append(persist.tile([P, seq], bf16, tag=f"qt{m}", name=f"qt{m}"))
            KT.append(persist.tile([P, seq], bf16, tag=f"kt{m}", name=f"kt{m}"))
        AT = []
        for m in range(2):
            AT.append(persist.tile([P, seq], bf16, tag=f"at{m}", name=f"at{m}"))

        with tc.tile_pool(name="xpool", bufs=1) as xpool, tc.tile_pool(
            name="projp", bufs=2, space="PSUM"
        ) as projp:

            def load_xt(xdram, name):
                t = xpool.tile([P, nd, seq], bf16, tag=name, name=name)
                h = nd // 2
                nc.sync.dma_start(
                    out=t[:, 0:h, :],
                    in_=xdram[: h * P, :].rearrange("(j p) s -> p j s", p=P),
                )
                nc.sync.dma_start(
                    out=t[:, h:nd, :],
                    in_=xdram[h * P :, :].rearrange("(j p) s -> p j s", p=P),
                )
                return t

            xq_t = load_xt(xq_d, "xq")
            xk_t = load_xt(xk_d, "xk")
            xv_t = load_xt(xv_d, "xv")

            def project_T(xt, wtile, res, name):
                ngroups = [
                    list(range(i, min(i + 2, nqc))) for i in range(0, nqc, 2)
                ]
                for m in range(2):
                    for gi, grp in enumerate(ngroups):
                        ps = projp.tile(
                            [P, len(grp) * QC],
                            f32,
                            tag="pj",
                            name=f"ps_{name}{m}_{gi}",
                        )
                        for half, n in enumerate(grp):
                            for j in range(nd):
                                nc.tensor.matmul(
                                    ps[:, half * QC : (half + 1) * QC],
                                    lhsT=wtile[
                                        :, j * DKB + m * P : j * DKB + (m + 1) * P
                                    ],
                                    rhs=xt[:, j, n * QC : (n + 1) * QC],
                                    start=(j == 0),
                                    stop=(j == nd - 1),
                                )
                        nc.vector.tensor_copy(
                            out=res[m][:, grp[0] * QC : (grp[-1] + 1) * QC],
                            in_=ps[:],
                        )

            project_T(xq_t, wq_t, QT, "qt")
            project_T(xk_t, wk_t, KT, "kt")

            for st in range(nkt):
                ps = projp.tile([P, DKB], f32, tag="pj", name=f"ps_v{st}")
                for j in range(nd):
                    nc.tensor.matmul(
                        ps[:],
                        lhsT=xv_t[:, j, st * P : (st + 1) * P],
                        rhs=wv_t[:, j * DKB : (j + 1) * DKB],
                        start=(j == 0),
                        stop=(j == nd - 1),
                    )
                v = persist.tile(
                    [P, HPC * (DK + 1)], bf16, tag=f"v{st}", name=f"v{st}"
                )
                nc.vector.memset(v[:], 1.0)
                nc.vector.tensor_copy(
                    out=v[:].rearrange("p (h w) -> p h w", w=DK + 1)[:, :, 0:DK],
                    in_=ps[:].rearrange("p (h w) -> p h w", w=DK),
                )
                vt.append(v)

        st_ps = ctx.enter_context(tc.tile_pool(name="st_ps", bufs=4, space="PSUM"))
        pv_ps = ctx.enter_context(tc.tile_pool(name="pv_ps", bufs=2, space="PSUM"))
        fp_ps = ctx.enter_context(tc.tile_pool(name="fp_ps", bufs=2, space="PSUM"))
        sc_pool = ctx.enter_context(tc.tile_pool(name="sc_pool", bufs=8))
        attn_pool = ctx.enter_context(tc.tile_pool(name="attn_pool", bufs=8))
        small = ctx.enter_context(tc.tile_pool(name="small", bufs=2))
        outp = ctx.enter_context(tc.tile_pool(name="outp", bufs=2))
        maskp = None
        if mask_mode == "full":
            maskp = ctx.enter_context(tc.tile_pool(name="maskp", bufs=2))

        exp_fn = mybir.ActivationFunctionType.Exp
        ln_fn = mybir.ActivationFunctionType.Ln
        for qc in range(nqc):
            mt = None
            if mask_mode == "full":
                mt = maskp.tile([P, nkt, QC], bf16, tag="mask", name=f"mt{qc}")
                nc.gpsimd.dma_start(
                    out=mt[:],
                    in_=maskt_d[:, qc * QC : (qc + 1) * QC].rearrange(
                        "(kt p) c -> p kt c", p=P
                    ),
                )
            for h in range(HPC):
                hm, hp = divmod(h, 2)
                hp *= DK
                kts = list(range(nkt))
                pairs = [kts[i : i + 2] for i in range(0, len(kts), 2)]

                pv = pv_ps.tile([DK + 1, QC], f32, tag="pv", name=f"pv{qc}_{h}")

                def emit_pv(at, pair, is_last):
                    for half, kt in enumerate(pair):
                        nc.tensor.matmul(
                            pv[:, 0:QC],
                            lhsT=vt[kt][:, h * (DK + 1) : (h + 1) * (DK + 1)],
                            rhs=at[:, half * QC : (half + 1) * QC],
                            start=(kt == 0),
                            stop=(is_last and half == len(pair) - 1),
                            skip_group_check=True,
                        )

                ats = []
                for pi, pair in enumerate(pairs):
                    sc = sc_pool.tile(
                        [P, 2 * QC], f32, tag="sc", name=f"sc{qc}_{h}_{pi}"
                    )
                    for half, kt in enumerate(pair):
                        stt = st_ps.tile(
                            [P, QC], f32, tag="st", name=f"st{qc}_{h}_{kt}"
                        )
                        nc.tensor.matmul(
                            stt[:, 0:QC],
                            lhsT=KT[hm][hp : hp + DK, kt * P : (kt + 1) * P],
                            rhs=QT[hm][hp : hp + DK, qc * QC : (qc + 1) * QC],
                            start=True,
                            stop=True,
                        )
                        dst = sc[:, half * QC : (half + 1) * QC]
                        if mask_mode == "full":
                            nc.vector.tensor_add(
                                out=dst, in0=stt[:, 0:QC], in1=mt[:, kt, :]
                            )
                        else:
                            nc.vector.tensor_copy(out=dst, in_=stt[:, 0:QC])
                    at = attn_pool.tile(
                        [P, 2 * QC], bf16, tag="attn", name=f"a{qc}_{h}_{pi}"
                    )
                    nc.scalar.activation(
                        out=at[:], in_=sc[:], func=exp_fn, scale=0.125
                    )
                    ats.append((at, pair))
                for at, pair in ats:
                    emit_pv(at, pair, pair is pairs[-1])
                lns = small.tile([1, QC], f32, tag="lns", name=f"ln{qc}_{h}")
                nc.scalar.activation(
                    out=lns[:], in_=pv[DK : DK + 1, :], func=ln_fn
                )
                bcp = fp_ps.tile([DK, QC], f32, tag="fp", name=f"bcp{qc}_{h}")
                nc.tensor.matmul(
                    bcp[:], lhsT=ones64[:], rhs=lns[:], start=True, stop=True
                )
                bc = small.tile([DK, QC], f32, tag="bcast", name=f"bc{qc}_{h}")
                nc.scalar.activation(
                    out=bc[:], in_=bcp[:], func=exp_fn, scale=-1.0
                )
                nc.vector.tensor_mul(
                    AT[hm][hp : hp + DK, qc * QC : (qc + 1) * QC],
                    pv[0:DK, :],
                    bc[:],
                )

            for j2 in range(qc * (QC // (2 * P)), (qc + 1) * (QC // (2 * P))):
                ob = outp.tile([P, 2, D], f32, tag="ob", name=f"ob{j2}")
                for g in range(2):
                    st = 2 * j2 + g
                    for nch in range(D // QC):
                        ps = fp_ps.tile(
                            [P, QC], f32, tag="fp", name=f"ps_o{st}_{nch}"
                        )
                        for m in range(2):
                            nc.tensor.matmul(
                                ps[:],
                                lhsT=AT[m][:, st * P : (st + 1) * P],
                                rhs=wo_t[
                                    :, m * D + nch * QC : m * D + (nch + 1) * QC
                                ],
                                start=(m == 0),
                                stop=(m == 1),
                            )
                        nc.vector.tensor_copy(
                            out=ob[:, g, nch * QC : (nch + 1) * QC], in_=ps[:]
                        )
                nc.sync.dma_start(
                    out=out_d[j2 * 2 * P : (j2 + 1) * 2 * P, :].rearrange(
                        "(g p) n -> p g n", p=P
                    ),
                    in_=ob[:],
                )

    return nc


def _split_multi_waits(nc):
    """This toolchain's walrus accepts at most one sync-wait per
    instruction. Hoist extra waits onto preceding same-engine NoOps —
    engine streams execute in order, so a NoOp that blocks on a
    semaphore gates everything after it (including HWDGE descriptor
    enqueues, which happen when the issuing engine's sequencer reaches
    the DMA instruction)."""
    import bass_rust

    ctr = 0
    for f in nc.m.functions:
        for bb in f.blocks:
            insts = bb.instructions
            new = []
            changed = False
            for inst in insts:
                si = inst.sync_info
                if si is not None and len(si.on_wait) > 1:
                    waits = list(si.on_wait)
                    for w in waits[:-1]:
                        ctr += 1
                        nop = bass_rust.InstNoOp(
                            name=f"wsplit_{ctr}", ins=[], outs=[]
                        )
                        nop.engine = inst.engine
                        nop.sync_info = bass_rust.SyncInfo(
                            on_wait=[w], on_update=[]
                        )
                        new.append(nop)
                    inst.sync_info = bass_rust.SyncInfo(
                        on_wait=[waits[-1]], on_update=list(si.on_update)
                    )
                    changed = True
                new.append(inst)
            if changed:
                try:
                    bb.instructions = new
                except AttributeError:
                    insts.clear()
                    insts.extend(new)
    return nc


def _get_nc(mask_mode, seq=S, split_waits=True):
    key = (mask_mode, seq, split_waits)
    if key not in _nc_cache:
        if mask_mode == "causal":
            nc = _build_causal(seq)
        else:
            nc = _build_legacy(mask_mode, seq)
        if split_waits:
            _split_multi_waits(nc)
        _nc_cache[key] = nc
    return _nc_cache[key]


def _pack_w(w_slice_T, ncols):
    # [D_in, ncols] -> [128, D_in/128 * ncols]: col block j holds rows j*128..
    d_in = w_slice_T.shape[0]
    return (
        w_slice_T.reshape(d_in // P, P, ncols).transpose(1, 0, 2).reshape(P, -1)
    )


def _tri_np():
    # preload = tri.T @ I : psum[p, j] = tri[j, p] = MASKVAL where j < p
    j = np.arange(P)[:, None]
    p = np.arange(P)[None, :]
    return np.where(j < p, np.float32(MASKVAL), np.float32(0.0)).astype(BF16)


def _detect_mask_mode(mask):
    if not mask.any():
        return "none"
    causal = np.triu(np.ones((mask.shape[1], mask.shape[2]), bool), k=1)
    if all(np.array_equal(mask[b], causal) for b in range(mask.shape[0])):
        return "causal"
    return "full"


def _make_in_maps(query, key, value, mask, w_q, w_k, w_v, w_o, mask_mode, seq=S):
    per_batch = []
    for b in range(B):
        d = {
            "xq_t": np.ascontiguousarray(query[b].T).astype(BF16),
            "xk_t": np.ascontiguousarray(key[b].T).astype(BF16),
            "xv_t": np.ascontiguousarray(value[b].T).astype(BF16),
        }
        if mask_mode == "full":
            d["mask_t"] = np.where(
                mask[b].T, np.float32(NEG), np.float32(0.0)
            ).astype(BF16)
        per_batch.append(d)
    per_hg = []
    for hg in range(HGROUPS):
        rows = slice(hg * DKB, (hg + 1) * DKB)
        wq_p = _pack_w(w_q[rows, :].T.astype(BF16), DKB)
        wk_p = _pack_w(w_k[rows, :].T.astype(BF16), DKB)
        wv_p = _pack_w(w_v[rows, :].T.astype(BF16), DKB)
        wo_p = _pack_w(w_o[:, rows].T.astype(BF16), D)
        if mask_mode == "causal":
            wrest = np.concatenate(
                [wk_p, wv_p, wo_p, _tri_np(), np.eye(P, dtype=BF16)], axis=1
            )
            per_hg.append({"wq_p": wq_p, "wrest": np.ascontiguousarray(wrest)})
        else:
            per_hg.append(
                {"wq_p": wq_p, "wk_p": wk_p, "wv_p": wv_p, "wo_p": wo_p}
            )
    in_maps = []
    for c in range(NCORE):
        b, hg = divmod(c, HGROUPS)
        im = dict(per_batch[b])
        im.update(per_hg[hg])
        in_maps.append(im)
    return in_maps


def _run(inputs, trace=False):
    from concourse.bass_utils import run_bass_kernel_spmd

    query = np.asarray(inputs["query"], np.float32)
    key = np.asarray(inputs["key"], np.float32)
    value = np.asarray(inputs["value"], np.float32)
    mask = np.asarray(inputs["mask"], bool)
    w_q = np.asarray(inputs["w_q"], np.float32)
    w_k = np.asarray(inputs["w_k"], np.float32)
    w_v = np.asarray(inputs["w_v"], np.float32)
    w_o = np.asarray(inputs["w_o"], np.float32)
    b_o = np.asarray(inputs["b_o"], np.float32)
    assert query.shape == (B, S, D), query.shape

    mask_mode = _detect_mask_mode(mask)
    nc = _get_nc(mask_mode)
    in_maps = _make_in_maps(query, key, value, mask, w_q, w_k, w_v, w_o, mask_mode)
    res = run_bass_kernel_spmd(nc, in_maps, list(range(NCORE)), trace=trace)
    outs = [np.asarray(r["out"], np.float32) for r in res.results]
    full = np.empty((B, S, D), np.float32)
    for b in range(B):
        full[b] = outs[HGROUPS * b]
        for i in range(1, HGROUPS):
            full[b] += outs[HGROUPS * b + i]
    full += b_o[None, None, :]
    return full, res


def kernel(**inputs):
    out, _ = _run(inputs, trace=False)
    return out


if __name__ == "__main__":
    import tempfile
    from concourse.bass_utils import compile_bass_kernel

    mode = sys.argv[1] if len(sys.argv) > 1 else "causal"
    nc = _get_nc(mode)
    from collections import Counter

    c = Counter()
    for name, inst in nc.inst_map.items():
        if "DMACopy" in type(inst).__name__:
            c[str(inst).count("wait:")] += 1
    print("DMA wait dist:", dict(c))
    td = tempfile.mkdtemp()
    p = compile_bass_kernel(nc, td)
    print("COMPILED OK:", p)


# revision 15
# speedup vs baseline: 1.3996x; 1.3996x over previous
"""Multi-head attention (B=2, S=2048, D=1024, H=16, causal mask) on 8 TRN2 cores.

Sharding: core c handles batch b = c // 4 and head-group hg = c % 4
(4 heads = 256 feature dims each). Each core computes its heads' QKV
projections, causal attention, and a partial output projection
(attn_out @ w_o[:, hg].T); the host sums the 4 partials per batch and
adds b_o.

Device layout (all chosen to avoid on-chip transposes):
  - host passes x.T [D, S] so projections contract d on partitions
  - Q,K kept transposed [dk, s]; V kept natural [s, dv] with a ones
    column appended -> PV matmul also emits softmax row sums
  - scores computed transposed S_T[k, q]; softmax without max
    subtraction (scores are O(+-8) for this data distribution)

Perf-critical structure (v3), from trace analysis of v1/v2:
  - the PE runs at 2.4 GHz only after ~10us of continuous execution and
    drops to 1.2 GHz when the stream has gaps, so the attention loop
    interleaves score-pairs of head-slot i with PV-pairs of slot i-1:
    the PE never waits on the scalar exp (which otherwise rate-matches
    the PE at ~0.83 ns/col vs 2x0.42)
  - input DMAs are chained 3-deep on 3 HWDGE queues in consumption
    order (wq->wk->wrest | xq->xk->xv): fair-share across parallel
    queues otherwise lands everything at ~45us, serializing all of
    phase 1 behind the DMA (v2: first score matmul at 83us). Queue
    assignment is a global round-robin over 8, so 5+5 tiny dummy DMAs
    pad the index space to keep each chain on one queue.
  - exp() reads score PSUM directly (v1 spent 54us of DVE copying
    PSUM->SBUF); causal diagonal masking is a constant 128x128
    triangular matrix matmul-preloaded into PSUM (start=True) that the
    score matmul accumulates onto (start=False)
  - softmax normalize: reciprocal_approx_fast (vector.reciprocal is
    3.3us per call - 51 ULP is plenty here), bf16 rank-1 matmul
    broadcast (1 cycle/col), DVE multiply; broadcast runs 1.5 slots
    late so its PSUM->SBUF staging never blocks the PE
  - output in bf16 (halves write traffic; host sums partials in fp32)
"""

import sys

if "/opt/trn_rl_repo" not in sys.path:
    sys.path.insert(0, "/opt/trn_rl_repo")

import numpy as np
import ml_dtypes

BF16 = ml_dtypes.bfloat16

B, S, D, H = 2, 2048, 1024, 16
NCORE = 8
HGROUPS = 4  # head-groups == cores per batch
HPC = H // HGROUPS  # heads per core = 4
DK = D // H  # head dim = 64
DKB = HPC * DK  # feature dims per core = 256
P = 128
QC = 512  # q chunk (one PSUM bank of fp32)
NEG = -1e9
MASKVAL = -30000.0  # exp(0.125*(s+MASKVAL)) == 0 for any realistic s

_nc_cache = {}


def _build_causal(seq=S):
    import concourse.bass as bass
    import concourse.tile as tile
    from concourse import mybir
    from contextlib import ExitStack

    f32 = mybir.dt.float32
    bf16 = mybir.dt.bfloat16
    exp_fn = mybir.ActivationFunctionType.Exp
    ln_fn = mybir.ActivationFunctionType.Ln
    nqc = seq // QC
    nkt = seq // P
    nd = D // P  # 8 d-chunks
    WREST = 2048 + 2 * P  # wo, tri, ident

    nc = bass.Bass()
    xq_d = nc.dram_tensor("xq_t", [D, seq], bf16, kind="ExternalInput")
    xk_d = nc.dram_tensor("xk_t", [D, seq], bf16, kind="ExternalInput")
    xv_d = nc.dram_tensor("xv_t", [D, seq], bf16, kind="ExternalInput")
    wq_d = nc.dram_tensor("wq_p", [P, D * DKB // P], bf16, kind="ExternalInput")
    wk_d = nc.dram_tensor("wk_p", [P, D * DKB // P], bf16, kind="ExternalInput")
    wv_d = nc.dram_tensor("wv_p", [P, D * DKB // P], bf16, kind="ExternalInput")
    wrest_d = nc.dram_tensor("wrest", [P, WREST], bf16, kind="ExternalInput")
    out_d = nc.dram_tensor("out", [seq, D], bf16, kind="ExternalOutput")

    with ExitStack() as ctx:
        tc = ctx.enter_context(tile.TileContext(nc))
        persist = ctx.enter_context(tc.tile_pool(name="persist", bufs=1))

        wrest_t = persist.tile([P, WREST], bf16, tag="wrest")
        wo_t = wrest_t[:, 0:2048]
        tri_t = wrest_t[:, 2048 : 2048 + P]
        idn_t = wrest_t[:, 2048 + P : 2048 + 2 * P]
        wq_t = persist.tile([P, D * DKB // P], bf16, tag="wq")
        wk_t = persist.tile([P, D * DKB // P], bf16, tag="wk")
        wv_t = persist.tile([P, D * DKB // P], bf16, tag="wv")
        xq_t = persist.tile([P, nd, seq], bf16, tag="xq", name="xq")
        xk_t = persist.tile([P, nd, seq], bf16, tag="xk", name="xk")
        xv_t = persist.tile([P, nd, seq], bf16, tag="xv", name="xv")

        ones64 = persist.tile([1, DK], bf16, tag="ones64")
        nc.vector.memset(ones64[:], 1.0)

        QT, KT, AT = [], [], []
        for m in range(2):
            QT.append(persist.tile([P, seq], bf16, tag=f"qt{m}", name=f"qt{m}"))
            KT.append(persist.tile([P, seq], bf16, tag=f"kt{m}", name=f"kt{m}"))
            AT.append(persist.tile([P, seq], bf16, tag=f"at{m}", name=f"at{m}"))
        vt = [
            persist.tile([P, HPC * (DK + 1)], bf16, tag=f"v{st}", name=f"v{st}")
            for st in range(nkt)
        ]

        # ---- DMA plan: emission index -> HWDGE queue is a global
        # round-robin mod 8, and per-queue throughput caps at ~60 GB/s, so
        # x tensors are striped as column-quarters across q0-q3 (chained
        # xq->xk->xv in consumption order) with the late-needed xq/xk
        # second halves on q6/q7 and weights chained on q4/q5.
        def load_cols(t, xdram, lo, hi):
            nc.sync.dma_start(
                out=t[:, :, lo:hi],
                in_=xdram[:, lo:hi].rearrange("(j p) s -> p j s", p=P),
            )

        hseq = seq // 2
        qtr = seq // 8  # 256-col pieces for the first half
        for i in range(4):  # idx0-3 -> q0-q3
            load_cols(xq_t, xq_d, i * qtr, (i + 1) * qtr)
        nc.sync.dma_start(out=wq_t[:], in_=wq_d[:, :])  # idx4  q4
        nc.sync.dma_start(out=wk_t[:], in_=wk_d[:, :])  # idx5  q5
        load_cols(xq_t, xq_d, hseq, hseq + seq // 4)  #   idx6  q6
        load_cols(xq_t, xq_d, hseq + seq // 4, seq)  #    idx7  q7
        for i in range(4):  # idx8-11 -> q0-q3
            load_cols(xk_t, xk_d, i * qtr, (i + 1) * qtr)
        nc.sync.dma_start(out=wv_t[:], in_=wv_d[:, :])  # idx12 q4
        nc.sync.dma_start(out=wrest_t[:], in_=wrest_d[:, :])  # idx13 q5
        load_cols(xk_t, xk_d, hseq, hseq + seq // 4)  #   idx14 q6
        load_cols(xk_t, xk_d, hseq + seq // 4, seq)  #    idx15 q7
        for i in range(4):  # idx16-19 -> q0-q3
            load_cols(xv_t, xv_d, i * qtr, (i + 1) * qtr)
        load_cols(xv_t, xv_d, hseq, hseq + seq // 4)  #   idx20 q4
        load_cols(xv_t, xv_d, hseq + seq // 4, seq)  #    idx21 q5
        # out DMAs (emitted later): idx22-25 -> q6,q7,q0,q1

        # ---- pools: st 2x2 banks + pv 2x1 + fp 2x1 = 8 PSUM banks ----
        st_ps = ctx.enter_context(tc.tile_pool(name="st_ps", bufs=2, space="PSUM"))
        pv_ps = ctx.enter_context(tc.tile_pool(name="pv_ps", bufs=2, space="PSUM"))
        fp_ps = ctx.enter_context(tc.tile_pool(name="fp_ps", bufs=2, space="PSUM"))
        attn_pool = ctx.enter_context(tc.tile_pool(name="attn_pool", bufs=11))
        lnf_pool = ctx.enter_context(tc.tile_pool(name="lnf_pool", bufs=2))
        hilo_pool = ctx.enter_context(tc.tile_pool(name="hilo_pool", bufs=4))
        binv_pool = ctx.enter_context(tc.tile_pool(name="binv_pool", bufs=2))
        outp = ctx.enter_context(tc.tile_pool(name="outp", bufs=2))

        # ---- projection units (one [P, QC] PSUM bank each; the first 12
        # run as a prologue, the rest inject into attention slots) ----
        def qk_unit(xt, wtile, res, m, c):
            ps = fp_ps.tile([P, QC], f32, tag="fp", name=f"pj{m}_{c}")
            for j in range(nd):
                nc.tensor.matmul(
                    ps[:],
                    lhsT=wtile[:, j * DKB + m * P : j * DKB + (m + 1) * P],
                    rhs=xt[:, j, c * QC : (c + 1) * QC],
                    start=(j == 0),
                    stop=(j == nd - 1),
                )
            nc.vector.tensor_copy(
                out=res[m][:, c * QC : (c + 1) * QC], in_=ps[:]
            )

        def v_unit(k0):
            ps = fp_ps.tile([P, QC], f32, tag="fp", name=f"pv_u{k0}")
            for half in range(2):
                kt = k0 + half
                for j in range(nd):
                    nc.tensor.matmul(
                        ps[:, half * DKB : half * DKB + DKB],
                        lhsT=xv_t[:, j, kt * P : (kt + 1) * P],
                        rhs=wv_t[:, j * DKB : (j + 1) * DKB],
                        start=(j == 0),
                        stop=(j == nd - 1),
                        skip_group_check=True,
                    )
            for half in range(2):
                kt = k0 + half
                v = vt[kt]
                nc.vector.memset(v[:], 1.0)
                nc.vector.tensor_copy(
                    out=v[:].rearrange("p (h w) -> p h w", w=DK + 1)[:, :, 0:DK],
                    in_=ps[:, half * DKB : half * DKB + DKB].rearrange(
                        "p (h w) -> p h w", w=DK
                    ),
                )

        # prologue: enough for qc0/qc1 attention (QT/KT cols 0:1024, vt 0-7)
        for c in range(2):
            for m in range(2):
                qk_unit(xq_t, wq_t, QT, m, c)
        for c in range(2):
            for m in range(2):
                qk_unit(xk_t, wk_t, KT, m, c)
        for k0 in range(0, 8, 2):
            v_unit(k0)
        # fillers: one list entry per attention slot boundary 1..11
        fillers = (
            [lambda m=m: qk_unit(xq_t, wq_t, QT, m, 2) for m in range(2)]
            + [lambda m=m: qk_unit(xk_t, wk_t, KT, m, 2) for m in range(2)]
            + [lambda k0=k0: v_unit(k0) for k0 in (8, 10)]
            + [lambda m=m: qk_unit(xq_t, wq_t, QT, m, 3) for m in range(2)]
            + [lambda m=m: qk_unit(xk_t, wk_t, KT, m, 3) for m in range(2)]
            + [lambda k0=k0: v_unit(k0) for k0 in (12, 14)]
        )
        inject = {s: [] for s in range(1, 12)}
        slot_for_unit = [1, 2, 3, 4, 5, 6, 6, 7, 8, 9, 10, 11]
        for u, s in zip(fillers, slot_for_unit):
            inject[s].append(u)

        def trim(kt, qc):
            o = kt * P - qc * QC
            return (o, True) if o >= 0 else (0, False)

        def make_pairs(qc):
            kts = list(range(4 * (qc + 1)))
            return [kts[i : i + 2] for i in range(0, len(kts), 2)]

        def score_pair(qc, h, pi, pair):
            """Score matmuls + exp for one kt-pair of (qc, h). Diagonal
            tiles get the triangular mask matmul-preloaded into PSUM; the
            exp span is merged across the pair (the dead gap columns hold
            stale PSUM whose exp lands in at columns PV never reads)."""
            hm, hp = divmod(h, 2)
            hp *= DK
            stt = st_ps.tile([P, 2 * QC], f32, tag="st", name=f"st{qc}_{h}_{pi}")
            for half, kt in enumerate(pair):
                o, diag = trim(kt, qc)
                b = half * QC
                kl = KT[hm][hp : hp + DK, kt * P : (kt + 1) * P]
                if diag:
                    nc.tensor.matmul(
                        stt[:, b + o : b + o + P],
                        lhsT=tri_t,
                        rhs=idn_t,
                        start=True,
                        stop=False,
                        skip_group_check=True,
                    )
                    nc.tensor.matmul(
                        stt[:, b + o : b + o + P],
                        lhsT=kl,
                        rhs=QT[hm][hp : hp + DK, qc * QC + o : qc * QC + o + P],
                        start=False,
                        stop=True,
                        skip_group_check=True,
                    )
                    if o + P < QC:
                        nc.tensor.matmul(
                            stt[:, b + o + P : b + QC],
                            lhsT=kl,
                            rhs=QT[hm][
                                hp : hp + DK, qc * QC + o + P : (qc + 1) * QC
                            ],
                            start=True,
                            stop=True,
                            skip_group_check=True,
                        )
                else:
                    nc.tensor.matmul(
                        stt[:, b : b + QC],
                        lhsT=kl,
                        rhs=QT[hm][hp : hp + DK, qc * QC : (qc + 1) * QC],
                        start=True,
                        stop=True,
                    )
            at = attn_pool.tile(
                [P, 2 * QC], bf16, tag="attn", name=f"a{qc}_{h}_{pi}"
            )
            o0 = trim(pair[0], qc)[0]
            nc.scalar.activation(
                out=at[:, o0 : 2 * QC],
                in_=stt[:, o0 : 2 * QC],
                func=exp_fn,
                scale=0.125,
            )
            return at

        def pv_pair(qc, h, pv, at, pair):
            last = 4 * (qc + 1) - 1
            for half, kt in enumerate(pair):
                o, _ = trim(kt, qc)
                nc.tensor.matmul(
                    pv[:, o:QC],
                    lhsT=vt[kt][:, h * (DK + 1) : (h + 1) * (DK + 1)],
                    rhs=at[:, half * QC + o : (half + 1) * QC],
                    start=(kt == 0),
                    stop=(kt == last),
                    skip_group_check=True,
                )

        def finish_recip(qc, h, pv):
            # 1/sums = exp(-ln(sums)); ln(sums) split hi+lo into two bf16
            # rows so the 1-cycle/col bf16 broadcast keeps fp32 accuracy
            lnf = lnf_pool.tile([1, QC], f32, tag="lnf", name=f"lf{qc}_{h}")
            nc.scalar.activation(out=lnf[:], in_=pv[DK : DK + 1, :], func=ln_fn)
            hi = hilo_pool.tile([1, QC], bf16, tag="lnhi", name=f"lh{qc}_{h}")
            nc.vector.tensor_copy(out=hi[:], in_=lnf[:])
            lo = hilo_pool.tile([1, QC], bf16, tag="lnlo", name=f"ll{qc}_{h}")
            nc.vector.tensor_sub(out=lo[:], in0=lnf[:], in1=hi[:])
            return (hi, lo)

        def emit_bcast_mul(qc, h, pv, inv):
            hi, lo = inv
            hm, hp = divmod(h, 2)
            hp *= DK
            bc = fp_ps.tile([DK, QC], f32, tag="fp", name=f"bc{qc}_{h}")
            nc.tensor.matmul(
                bc[:], lhsT=ones64[:], rhs=hi[:], start=True, stop=False
            )
            nc.tensor.matmul(
                bc[:],
                lhsT=ones64[:],
                rhs=lo[:],
                start=False,
                stop=True,
                skip_group_check=True,
            )
            binv = binv_pool.tile([DK, QC], f32, tag="binv", name=f"bi{qc}_{h}")
            nc.scalar.activation(out=binv[:], in_=bc[:], func=exp_fn, scale=-1.0)
            nc.vector.tensor_mul(
                AT[hm][hp : hp + DK, qc * QC : (qc + 1) * QC],
                pv[0:DK, :],
                binv[:],
            )

        def emit_outproj(qc):
            ob = outp.tile([P, 4, D], bf16, tag="ob", name=f"ob{qc}")
            for g in range(4):
                st = 4 * qc + g
                for nch in range(D // QC):
                    ps = fp_ps.tile([P, QC], f32, tag="fp", name=f"po{st}_{nch}")
                    for m in range(2):
                        nc.tensor.matmul(
                            ps[:],
                            lhsT=AT[m][:, st * P : (st + 1) * P],
                            rhs=wo_t[:, m * D + nch * QC : m * D + (nch + 1) * QC],
                            start=(m == 0),
                            stop=(m == 1),
                        )
                    nc.vector.tensor_copy(
                        out=ob[:, g, nch * QC : (nch + 1) * QC], in_=ps[:]
                    )
            nc.sync.dma_start(
                out=out_d[qc * QC : (qc + 1) * QC, :].rearrange(
                    "(g p) n -> p g n", p=P
                ),
                in_=ob[:],
            )

        # flat head-slot pipeline: score-pairs of slot i interleave with
        # PV-pairs of slot i-1; normalize trails by ~1.5 slots; projection
        # filler units and the previous qc's out-projection keep the PE fed
        slots = [(qc, h) for qc in range(nqc) for h in range(HPC)]
        prev = None  # (qc, h, ats)
        pend_bc = []
        for si, (qc, h) in enumerate(slots):
            for u in inject.get(si, []):
                u()
            pairs_now = make_pairs(qc)
            ats_now = []
            pv_prev = None
            if prev is not None:
                pv_prev = pv_ps.tile(
                    [DK + 1, QC], f32, tag="pv", name=f"pv{prev[0]}_{prev[1]}"
                )
            for pi, pair in enumerate(pairs_now):
                ats_now.append((score_pair(qc, h, pi, pair), pair))
                if prev is not None and pi < len(prev[2]):
                    pat, ppair = prev[2][pi]
                    pv_pair(prev[0], prev[1], pv_prev, pat, ppair)
                if pi == 1 and pend_bc:
                    emit_bcast_mul(*pend_bc.pop(0))
                # (qc-1, h3)'s normalize pops at slot (qc, h1) pi==1, so
                # the out-projection of qc-1 is safe only from pi==3 here
                if pi == 3 and h == 1 and qc > 0:
                    emit_outproj(qc - 1)
            if prev is not None:
                inv = finish_recip(prev[0], prev[1], pv_prev)
                pend_bc.append((prev[0], prev[1], pv_prev, inv))
            prev = (qc, h, ats_now)
        # tail: PV + normalize for the last slot, then the last outproj
        pv_last = pv_ps.tile([DK + 1, QC], f32, tag="pv", name="pv_last")
        for pi, (at, pair) in enumerate(prev[2]):
            pv_pair(prev[0], prev[1], pv_last, at, pair)
            if pi == 1 and pend_bc:
                emit_bcast_mul(*pend_bc.pop(0))
        inv = finish_recip(prev[0], prev[1], pv_last)
        pend_bc.append((prev[0], prev[1], pv_last, inv))
        while pend_bc:
            emit_bcast_mul(*pend_bc.pop(0))
        emit_outproj(nqc - 1)

    return nc


def _build_legacy(mask_mode, seq=S):
    """Fallback for non-causal masks (mask_mode: 'none'|'full')."""
    import concourse.bass as bass
    import concourse.tile as tile
    from concourse import mybir
    from contextlib import ExitStack

    f32 = mybir.dt.float32
    bf16 = mybir.dt.bfloat16
    nqc = seq // QC
    nkt = seq // P
    nd = D // P  # 8 d-chunks

    nc = bass.Bass(num_swdge_queues=4)
    xq_d = nc.dram_tensor("xq_t", [D, seq], bf16, kind="ExternalInput")
    xk_d = nc.dram_tensor("xk_t", [D, seq], bf16, kind="ExternalInput")
    xv_d = nc.dram_tensor("xv_t", [D, seq], bf16, kind="ExternalInput")
    wq_d = nc.dram_tensor("wq_p", [P, D * DKB // P], bf16, kind="ExternalInput")
    wk_d = nc.dram_tensor("wk_p", [P, D * DKB // P], bf16, kind="ExternalInput")
    wv_d = nc.dram_tensor("wv_p", [P, D * DKB // P], bf16, kind="ExternalInput")
    wo_d = nc.dram_tensor("wo_p", [P, DKB * D // P], bf16, kind="ExternalInput")
    if mask_mode == "full":
        maskt_d = nc.dram_tensor("mask_t", [seq, seq], bf16, kind="ExternalInput")
    out_d = nc.dram_tensor("out", [seq, D], f32, kind="ExternalOutput")

    with ExitStack() as ctx:
        tc = ctx.enter_context(tile.TileContext(nc))
        persist = ctx.enter_context(tc.tile_pool(name="persist", bufs=1))

        ones64 = persist.tile([1, DK], f32, tag="ones64")
        nc.vector.memset(ones64[:], 1.0)
        wq_t = persist.tile([P, D * DKB // P], bf16, tag="wq")
        wk_t = persist.tile([P, D * DKB // P], bf16, tag="wk")
        wv_t = persist.tile([P, D * DKB // P], bf16, tag="wv")
        wo_t = persist.tile([P, DKB * D // P], bf16, tag="wo")
        nc.gpsimd.dma_start(out=wq_t[:], in_=wq_d[:, :])
        nc.gpsimd.dma_start(out=wk_t[:], in_=wk_d[:, :])
        nc.gpsimd.dma_start(out=wv_t[:], in_=wv_d[:, :])
        nc.gpsimd.dma_start(out=wo_t[:], in_=wo_d[:, :])

        QT, KT, vt = [], [], []
        for m in range(2):
            QT.append(persist.tile([P, seq], bf16, tag=f"qt{m}", name=f"qt{m}"))
            KT.append(persist.tile([P, seq], bf16, tag=f"kt{m}", name=f"kt{m}"))
        AT = []
        for m in range(2):
            AT.append(persist.tile([P, seq], bf16, tag=f"at{m}", name=f"at{m}"))

        with tc.tile_pool(name="xpool", bufs=1) as xpool, tc.tile_pool(
            name="projp", bufs=2, space="PSUM"
        ) as projp:

            def load_xt(xdram, name):
                t = xpool.tile([P, nd, seq], bf16, tag=name, name=name)
                h = nd // 2
                nc.sync.dma_start(
                    out=t[:, 0:h, :],
                    in_=xdram[: h * P, :].rearrange("(j p) s -> p j s", p=P),
                )
                nc.sync.dma_start(
                    out=t[:, h:nd, :],
                    in_=xdram[h * P :, :].rearrange("(j p) s -> p j s", p=P),
                )
                return t

            xq_t = load_xt(xq_d, "xq")
            xk_t = load_xt(xk_d, "xk")
            xv_t = load_xt(xv_d, "xv")

            def project_T(xt, wtile, res, name):
                ngroups = [
                    list(range(i, min(i + 2, nqc))) for i in range(0, nqc, 2)
                ]
                for m in range(2):
                    for gi, grp in enumerate(ngroups):
                        ps = projp.tile(
                            [P, len(grp) * QC],
                            f32,
                            tag="pj",
                            name=f"ps_{name}{m}_{gi}",
                        )
                        for half, n in enumerate(grp):
                            for j in range(nd):
                                nc.tensor.matmul(
                                    ps[:, half * QC : (half + 1) * QC],
                                    lhsT=wtile[
                                        :, j * DKB + m * P : j * DKB + (m + 1) * P
                                    ],
                                    rhs=xt[:, j, n * QC : (n + 1) * QC],
                                    start=(j == 0),
                                    stop=(j == nd - 1),
                                )
                        nc.vector.tensor_copy(
                            out=res[m][:, grp[0] * QC : (grp[-1] + 1) * QC],
                            in_=ps[:],
                        )

            project_T(xq_t, wq_t, QT, "qt")
            project_T(xk_t, wk_t, KT, "kt")

            for st in range(nkt):
                ps = projp.tile([P, DKB], f32, tag="pj", name=f"ps_v{st}")
                for j in range(nd):
                    nc.tensor.matmul(
                        ps[:],
                        lhsT=xv_t[:, j, st * P : (st + 1) * P],
                        rhs=wv_t[:, j * DKB : (j + 1) * DKB],
                        start=(j == 0),
                        stop=(j == nd - 1),
                    )
                v = persist.tile(
                    [P, HPC * (DK + 1)], bf16, tag=f"v{st}", name=f"v{st}"
                )
                nc.vector.memset(v[:], 1.0)
                nc.vector.tensor_copy(
                    out=v[:].rearrange("p (h w) -> p h w", w=DK + 1)[:, :, 0:DK],
                    in_=ps[:].rearrange("p (h w) -> p h w", w=DK),
                )
                vt.append(v)

        st_ps = ctx.enter_context(tc.tile_pool(name="st_ps", bufs=4, space="PSUM"))
        pv_ps = ctx.enter_context(tc.tile_pool(name="pv_ps", bufs=2, space="PSUM"))
        fp_ps = ctx.enter_context(tc.tile_pool(name="fp_ps", bufs=2, space="PSUM"))
        sc_pool = ctx.enter_context(tc.tile_pool(name="sc_pool", bufs=8))
        attn_pool = ctx.enter_context(tc.tile_pool(name="attn_pool", bufs=8))
        small = ctx.enter_context(tc.tile_pool(name="small", bufs=2))
        outp = ctx.enter_context(tc.tile_pool(name="outp", bufs=2))
        maskp = None
        if mask_mode == "full":
            maskp = ctx.enter_context(tc.tile_pool(name="maskp", bufs=2))

        exp_fn = mybir.ActivationFunctionType.Exp
        ln_fn = mybir.ActivationFunctionType.Ln
        for qc in range(nqc):
            mt = None
            if mask_mode == "full":
                mt = maskp.tile([P, nkt, QC], bf16, tag="mask", name=f"mt{qc}")
                nc.gpsimd.dma_start(
                    out=mt[:],
                    in_=maskt_d[:, qc * QC : (qc + 1) * QC].rearrange(
                        "(kt p) c -> p kt c", p=P
                    ),
                )
            for h in range(HPC):
                hm, hp = divmod(h, 2)
                hp *= DK
                kts = list(range(nkt))
                pairs = [kts[i : i + 2] for i in range(0, len(kts), 2)]

                pv = pv_ps.tile([DK + 1, QC], f32, tag="pv", name=f"pv{qc}_{h}")

                def emit_pv(at, pair, is_last):
                    for half, kt in enumerate(pair):
                        nc.tensor.matmul(
                            pv[:, 0:QC],
                            lhsT=vt[kt][:, h * (DK + 1) : (h + 1) * (DK + 1)],
                            rhs=at[:, half * QC : (half + 1) * QC],
                            start=(kt == 0),
                            stop=(is_last and half == len(pair) - 1),
                            skip_group_check=True,
                        )

                ats = []
                for pi, pair in enumerate(pairs):
                    sc = sc_pool.tile(
                        [P, 2 * QC], f32, tag="sc", name=f"sc{qc}_{h}_{pi}"
                    )
                    for half, kt in enumerate(pair):
                        stt = st_ps.tile(
                            [P, QC], f32, tag="st", name=f"st{qc}_{h}_{kt}"
                        )
                        nc.tensor.matmul(
                            stt[:, 0:QC],
                            lhsT=KT[hm][hp : hp + DK, kt * P : (kt + 1) * P],
                            rhs=QT[hm][hp : hp + DK, qc * QC : (qc + 1) * QC],
                            start=True,
                            stop=True,
                        )
                        dst = sc[:, half * QC : (half + 1) * QC]
                        if mask_mode == "full":
                            nc.vector.tensor_add(
                                out=dst, in0=stt[:, 0:QC], in1=mt[:, kt, :]
                            )
                        else:
                            nc.vector.tensor_copy(out=dst, in_=stt[:, 0:QC])
                    at = attn_pool.tile(
                        [P, 2 * QC], bf16, tag="attn", name=f"a{qc}_{h}_{pi}"
                    )
                    nc.scalar.activation(
                        out=at[:], in_=sc[:], func=exp_fn, scale=0.125
                    )
                    ats.append((at, pair))
                for at, pair in ats:
                    emit_pv(at, pair, pair is pairs[-1])
                lns = small.tile([1, QC], f32, tag="lns", name=f"ln{qc}_{h}")
                nc.scalar.activation(
                    out=lns[:], in_=pv[DK : DK + 1, :], func=ln_fn
                )
                bcp = fp_ps.tile([DK, QC], f32, tag="fp", name=f"bcp{qc}_{h}")
                nc.tensor.matmul(
                    bcp[:], lhsT=ones64[:], rhs=lns[:], start=True, stop=True
                )
                bc = small.tile([DK, QC], f32, tag="bcast", name=f"bc{qc}_{h}")
                nc.scalar.activation(
                    out=bc[:], in_=bcp[:], func=exp_fn, scale=-1.0
                )
                nc.vector.tensor_mul(
                    AT[hm][hp : hp + DK, qc * QC : (qc + 1) * QC],
                    pv[0:DK, :],
                    bc[:],
                )

            for j2 in range(qc * (QC // (2 * P)), (qc + 1) * (QC // (2 * P))):
                ob = outp.tile([P, 2, D], f32, tag="ob", name=f"ob{j2}")
                for g in range(2):
                    st = 2 * j2 + g
                    for nch in range(D // QC):
                        ps = fp_ps.tile(
                            [P, QC], f32, tag="fp", name=f"ps_o{st}_{nch}"
                        )
                        for m in range(2):
                            nc.tensor.matmul(
                                ps[:],
                                lhsT=AT[m][:, st * P : (st + 1) * P],
                                rhs=wo_t[
                                    :, m * D + nch * QC : m * D + (nch + 1) * QC
                                ],
                                start=(m == 0),
                                stop=(m == 1),
                            )
                        nc.vector.tensor_copy(
                            out=ob[:, g, nch * QC : (nch + 1) * QC], in_=ps[:]
                        )
                nc.sync.dma_start(
                    out=out_d[j2 * 2 * P : (j2 + 1) * 2 * P, :].rearrange(
                        "(g p) n -> p g n", p=P
                    ),
                    in_=ob[:],
                )

    return nc


def _split_multi_waits(nc):
    """This toolchain's walrus accepts at most one sync-wait per
    instruction. Hoist extra waits onto preceding same-engine NoOps —
    engine streams execute in order, so a NoOp that blocks on a
    semaphore gates everything after it (including HWDGE descriptor
    enqueues, which happen when the issuing engine's sequencer reaches
    the DMA instruction)."""
    import bass_rust

    ctr = 0
    for f in nc.m.functions:
        for bb in f.blocks:
            insts = bb.instructions
            new = []
            changed = False
            for inst in insts:
                si = inst.sync_info
                if si is not None and len(si.on_wait) > 1:
                    waits = list(si.on_wait)
                    for w in waits[:-1]:
                        ctr += 1
                        nop = bass_rust.InstNoOp(
                            name=f"wsplit_{ctr}", ins=[], outs=[]
                        )
                        nop.engine = inst.engine
                        nop.sync_info = bass_rust.SyncInfo(
                            on_wait=[w], on_update=[]
                        )
                        new.append(nop)
                    inst.sync_info = bass_rust.SyncInfo(
                        on_wait=[waits[-1]], on_update=list(si.on_update)
                    )
                    changed = True
                new.append(inst)
            if changed:
                try:
                    bb.instructions = new
                except AttributeError:
                    insts.clear()
                    insts.extend(new)
    return nc


def _get_nc(mask_mode, seq=S, split_waits=True):
    key = (mask_mode, seq, split_waits)
    if key not in _nc_cache:
        if mask_mode == "causal":
            nc = _build_causal(seq)
        else:
            nc = _build_legacy(mask_mode, seq)
        if split_waits:
            _split_multi_waits(nc)
        _nc_cache[key] = nc
    return _nc_cache[key]


def _pack_w(w_slice_T, ncols):
    # [D_in, ncols] -> [128, D_in/128 * ncols]: col block j holds rows j*128..
    d_in = w_slice_T.shape[0]
    return (
        w_slice_T.reshape(d_in // P, P, ncols).transpose(1, 0, 2).reshape(P, -1)
    )


def _tri_np():
    # preload = tri.T @ I : psum[p, j] = tri[j, p] = MASKVAL where j < p
    j = np.arange(P)[:, None]
    p = np.arange(P)[None, :]
    return np.where(j < p, np.float32(MASKVAL), np.float32(0.0)).astype(BF16)


def _detect_mask_mode(mask):
    if not mask.any():
        return "none"
    causal = np.triu(np.ones((mask.shape[1], mask.shape[2]), bool), k=1)
    if all(np.array_equal(mask[b], causal) for b in range(mask.shape[0])):
        return "causal"
    return "full"


def _make_in_maps(query, key, value, mask, w_q, w_k, w_v, w_o, mask_mode, seq=S):
    per_batch = []
    for b in range(B):
        d = {
            "xq_t": np.ascontiguousarray(query[b].T).astype(BF16),
            "xk_t": np.ascontiguousarray(key[b].T).astype(BF16),
            "xv_t": np.ascontiguousarray(value[b].T).astype(BF16),
        }
        if mask_mode == "full":
            d["mask_t"] = np.where(
                mask[b].T, np.float32(NEG), np.float32(0.0)
            ).astype(BF16)
        per_batch.append(d)
    per_hg = []
    for hg in range(HGROUPS):
        rows = slice(hg * DKB, (hg + 1) * DKB)
        wq_p = _pack_w(w_q[rows, :].T.astype(BF16), DKB)
        wk_p = _pack_w(w_k[rows, :].T.astype(BF16), DKB)
        wv_p = _pack_w(w_v[rows, :].T.astype(BF16), DKB)
        wo_p = _pack_w(w_o[:, rows].T.astype(BF16), D)
        if mask_mode == "causal":
            wrest = np.concatenate(
                [wo_p, _tri_np(), np.eye(P, dtype=BF16)], axis=1
            )
            per_hg.append(
                {
                    "wq_p": wq_p,
                    "wk_p": wk_p,
                    "wv_p": wv_p,
                    "wrest": np.ascontiguousarray(wrest),
                }
            )
        else:
            per_hg.append(
                {"wq_p": wq_p, "wk_p": wk_p, "wv_p": wv_p, "wo_p": wo_p}
            )
    in_maps = []
    for c in range(NCORE):
        b, hg = divmod(c, HGROUPS)
        im = dict(per_batch[b])
        im.update(per_hg[hg])
        in_maps.append(im)
    return in_maps


def _run(inputs, trace=False):
    from concourse.bass_utils import run_bass_kernel_spmd

    query = np.asarray(inputs["query"], np.float32)
    key = np.asarray(inputs["key"], np.float32)
    value = np.asarray(inputs["value"], np.float32)
    mask = np.asarray(inputs["mask"], bool)
    w_q = np.asarray(inputs["w_q"], np.float32)
    w_k = np.asarray(inputs["w_k"], np.float32)
    w_v = np.asarray(inputs["w_v"], np.float32)
    w_o = np.asarray(inputs["w_o"], np.float32)
    b_o = np.asarray(inputs["b_o"], np.float32)
    assert query.shape == (B, S, D), query.shape

    mask_mode = _detect_mask_mode(mask)
    nc = _get_nc(mask_mode)
    in_maps = _make_in_maps(query, key, value, mask, w_q, w_k, w_v, w_o, mask_mode)
    res = run_bass_kernel_spmd(nc, in_maps, list(range(NCORE)), trace=trace)
    outs = [np.asarray(r["out"], np.float32) for r in res.results]
    full = np.empty((B, S, D), np.float32)
    for b in range(B):
        full[b] = outs[HGROUPS * b]
        for i in range(1, HGROUPS):
            full[b] += outs[HGROUPS * b + i]
    full += b_o[None, None, :]
    return full, res


def kernel(**inputs):
    out, _ = _run(inputs, trace=False)
    return out


if __name__ == "__main__":
    import tempfile
    from concourse.bass_utils import compile_bass_kernel

    mode = sys.argv[1] if len(sys.argv) > 1 else "causal"
    nc = _get_nc(mode)
    from collections import Counter

    c = Counter()
    for name, inst in nc.inst_map.items():
        if "DMACopy" in type(inst).__name__:
            c[str(inst).count("wait:")] += 1
    print("DMA wait dist:", dict(c))
    td = tempfile.mkdtemp()
    p = compile_bass_kernel(nc, td)
    print("COMPILED OK:", p)


# revision 16
# speedup vs baseline: 1.4206x; 1.0150x over previous
"""Multi-head attention (B=2, S=2048, D=1024, H=16, causal mask) on 8 TRN2 cores.

Sharding: core c handles batch b = c // 4 and head-group hg = c % 4
(4 heads = 256 feature dims each). Each core computes its heads' QKV
projections, causal attention, and a partial output projection
(attn_out @ w_o[:, hg].T); the host sums the 4 partials per batch and
adds b_o.

Device layout (all chosen to avoid on-chip transposes):
  - host passes x.T [D, S] so projections contract d on partitions
  - Q,K kept transposed [dk, s]; V kept natural [s, dv] with a ones
    column appended -> PV matmul also emits softmax row sums
  - scores computed transposed S_T[k, q]; softmax without max
    subtraction (scores are O(+-8) for this data distribution)

Perf-critical structure (v3), from trace analysis of v1/v2:
  - the PE runs at 2.4 GHz only after ~10us of continuous execution and
    drops to 1.2 GHz when the stream has gaps, so the attention loop
    interleaves score-pairs of head-slot i with PV-pairs of slot i-1:
    the PE never waits on the scalar exp (which otherwise rate-matches
    the PE at ~0.83 ns/col vs 2x0.42)
  - input DMAs are chained 3-deep on 3 HWDGE queues in consumption
    order (wq->wk->wrest | xq->xk->xv): fair-share across parallel
    queues otherwise lands everything at ~45us, serializing all of
    phase 1 behind the DMA (v2: first score matmul at 83us). Queue
    assignment is a global round-robin over 8, so 5+5 tiny dummy DMAs
    pad the index space to keep each chain on one queue.
  - exp() reads score PSUM directly (v1 spent 54us of DVE copying
    PSUM->SBUF); causal diagonal masking is a constant 128x128
    triangular matrix matmul-preloaded into PSUM (start=True) that the
    score matmul accumulates onto (start=False)
  - softmax normalize: reciprocal_approx_fast (vector.reciprocal is
    3.3us per call - 51 ULP is plenty here), bf16 rank-1 matmul
    broadcast (1 cycle/col), DVE multiply; broadcast runs 1.5 slots
    late so its PSUM->SBUF staging never blocks the PE
  - output in bf16 (halves write traffic; host sums partials in fp32)
"""

import sys

if "/opt/trn_rl_repo" not in sys.path:
    sys.path.insert(0, "/opt/trn_rl_repo")

import numpy as np
import ml_dtypes

BF16 = ml_dtypes.bfloat16

B, S, D, H = 2, 2048, 1024, 16
NCORE = 8
HGROUPS = 4  # head-groups == cores per batch
HPC = H // HGROUPS  # heads per core = 4
DK = D // H  # head dim = 64
DKB = HPC * DK  # feature dims per core = 256
P = 128
QC = 512  # q chunk (one PSUM bank of fp32)
NEG = -1e9
MASKVAL = -30000.0  # exp(0.125*(s+MASKVAL)) == 0 for any realistic s

_nc_cache = {}


def _build_causal(seq=S):
    import concourse.bass as bass
    import concourse.tile as tile
    from concourse import mybir
    from contextlib import ExitStack

    f32 = mybir.dt.float32
    bf16 = mybir.dt.bfloat16
    exp_fn = mybir.ActivationFunctionType.Exp
    ln_fn = mybir.ActivationFunctionType.Ln
    nqc = seq // QC
    nkt = seq // P
    nd = D // P  # 8 d-chunks
    WREST = 2048 + 2 * P  # wo, tri, ident

    nc = bass.Bass()
    xq_d = nc.dram_tensor("xq_t", [D, seq], bf16, kind="ExternalInput")
    xk_d = nc.dram_tensor("xk_t", [D, seq], bf16, kind="ExternalInput")
    xv_d = nc.dram_tensor("xv_t", [D, seq], bf16, kind="ExternalInput")
    wq_d = nc.dram_tensor("wq_p", [P, D * DKB // P], bf16, kind="ExternalInput")
    wk_d = nc.dram_tensor("wk_p", [P, D * DKB // P], bf16, kind="ExternalInput")
    wv_d = nc.dram_tensor("wv_p", [P, D * DKB // P], bf16, kind="ExternalInput")
    wrest_d = nc.dram_tensor("wrest", [P, WREST], bf16, kind="ExternalInput")
    out_d = nc.dram_tensor("out", [seq, D], bf16, kind="ExternalOutput")

    with ExitStack() as ctx:
        tc = ctx.enter_context(tile.TileContext(nc))
        persist = ctx.enter_context(tc.tile_pool(name="persist", bufs=1))

        wrest_t = persist.tile([P, WREST], bf16, tag="wrest")
        wo_t = wrest_t[:, 0:2048]
        tri_t = wrest_t[:, 2048 : 2048 + P]
        idn_t = wrest_t[:, 2048 + P : 2048 + 2 * P]
        wq_t = persist.tile([P, D * DKB // P], bf16, tag="wq")
        wk_t = persist.tile([P, D * DKB // P], bf16, tag="wk")
        wv_t = persist.tile([P, D * DKB // P], bf16, tag="wv")
        xq_t = persist.tile([P, nd, seq], bf16, tag="xq", name="xq")
        xk_t = persist.tile([P, nd, seq], bf16, tag="xk", name="xk")
        xv_t = persist.tile([P, nd, seq], bf16, tag="xv", name="xv")

        ones64 = persist.tile([1, DK], bf16, tag="ones64")
        nc.vector.memset(ones64[:], 1.0)

        QT, KT, AT = [], [], []
        for m in range(2):
            QT.append(persist.tile([P, seq], bf16, tag=f"qt{m}", name=f"qt{m}"))
            KT.append(persist.tile([P, seq], bf16, tag=f"kt{m}", name=f"kt{m}"))
            AT.append(persist.tile([P, seq], bf16, tag=f"at{m}", name=f"at{m}"))
        vt = [
            persist.tile([P, HPC * (DK + 1)], bf16, tag=f"v{st}", name=f"v{st}")
            for st in range(nkt)
        ]

        # ---- DMA plan: emission index -> HWDGE queue is a global
        # round-robin mod 8, and per-queue throughput caps at ~60 GB/s, so
        # x tensors are striped as column-quarters across q0-q3 (chained
        # xq->xk->xv in consumption order) with the late-needed xq/xk
        # second halves on q6/q7 and weights chained on q4/q5.
        def load_cols(t, xdram, lo, hi):
            nc.sync.dma_start(
                out=t[:, :, lo:hi],
                in_=xdram[:, lo:hi].rearrange("(j p) s -> p j s", p=P),
            )

        hseq = seq // 2
        qtr = seq // 8  # 256-col pieces for the first half
        for i in range(4):  # idx0-3 -> q0-q3
            load_cols(xq_t, xq_d, i * qtr, (i + 1) * qtr)
        nc.sync.dma_start(out=wq_t[:], in_=wq_d[:, :])  # idx4  q4
        nc.sync.dma_start(out=wk_t[:], in_=wk_d[:, :])  # idx5  q5
        load_cols(xq_t, xq_d, hseq, hseq + seq // 4)  #   idx6  q6
        load_cols(xq_t, xq_d, hseq + seq // 4, seq)  #    idx7  q7
        for i in range(4):  # idx8-11 -> q0-q3
            load_cols(xk_t, xk_d, i * qtr, (i + 1) * qtr)
        nc.sync.dma_start(out=wv_t[:], in_=wv_d[:, :])  # idx12 q4
        nc.sync.dma_start(out=wrest_t[:], in_=wrest_d[:, :])  # idx13 q5
        load_cols(xk_t, xk_d, hseq, hseq + seq // 4)  #   idx14 q6
        load_cols(xk_t, xk_d, hseq + seq // 4, seq)  #    idx15 q7
        for i in range(4):  # idx16-19 -> q0-q3
            load_cols(xv_t, xv_d, i * qtr, (i + 1) * qtr)
        load_cols(xv_t, xv_d, hseq, hseq + seq // 4)  #   idx20 q4
        load_cols(xv_t, xv_d, hseq + seq // 4, seq)  #    idx21 q5
        # out DMAs (emitted later): idx22-25 -> q6,q7,q0,q1

        # ---- pools: st 2x2 banks + pv 2x1 + fp 2x1 = 8 PSUM banks ----
        st_ps = ctx.enter_context(tc.tile_pool(name="st_ps", bufs=2, space="PSUM"))
        pv_ps = ctx.enter_context(tc.tile_pool(name="pv_ps", bufs=2, space="PSUM"))
        fp_ps = ctx.enter_context(tc.tile_pool(name="fp_ps", bufs=2, space="PSUM"))
        attn_pool = ctx.enter_context(tc.tile_pool(name="attn_pool", bufs=11))
        lnf_pool = ctx.enter_context(tc.tile_pool(name="lnf_pool", bufs=2))
        hilo_pool = ctx.enter_context(tc.tile_pool(name="hilo_pool", bufs=4))
        binv_pool = ctx.enter_context(tc.tile_pool(name="binv_pool", bufs=2))
        outp = ctx.enter_context(tc.tile_pool(name="outp", bufs=2))

        # ---- projection units (one [P, QC] PSUM bank each; the first 12
        # run as a prologue, the rest inject into attention slots) ----
        def qk_unit(xt, wtile, res, m, c):
            ps = fp_ps.tile([P, QC], f32, tag="fp", name=f"pj{m}_{c}")
            for j in range(nd):
                nc.tensor.matmul(
                    ps[:],
                    lhsT=wtile[:, j * DKB + m * P : j * DKB + (m + 1) * P],
                    rhs=xt[:, j, c * QC : (c + 1) * QC],
                    start=(j == 0),
                    stop=(j == nd - 1),
                )
            nc.vector.tensor_copy(
                out=res[m][:, c * QC : (c + 1) * QC], in_=ps[:]
            )

        def v_unit(k0):
            ps = fp_ps.tile([P, QC], f32, tag="fp", name=f"pv_u{k0}")
            for half in range(2):
                kt = k0 + half
                for j in range(nd):
                    nc.tensor.matmul(
                        ps[:, half * DKB : half * DKB + DKB],
                        lhsT=xv_t[:, j, kt * P : (kt + 1) * P],
                        rhs=wv_t[:, j * DKB : (j + 1) * DKB],
                        start=(j == 0),
                        stop=(j == nd - 1),
                        skip_group_check=True,
                    )
            for half in range(2):
                kt = k0 + half
                v = vt[kt]
                nc.vector.memset(v[:], 1.0)
                nc.vector.tensor_copy(
                    out=v[:].rearrange("p (h w) -> p h w", w=DK + 1)[:, :, 0:DK],
                    in_=ps[:, half * DKB : half * DKB + DKB].rearrange(
                        "p (h w) -> p h w", w=DK
                    ),
                )

        # prologue: enough for qc0/qc1 attention (QT/KT cols 0:1024, vt 0-7)
        for c in range(2):
            for m in range(2):
                qk_unit(xq_t, wq_t, QT, m, c)
        for c in range(2):
            for m in range(2):
                qk_unit(xk_t, wk_t, KT, m, c)
        for k0 in range(0, 8, 2):
            v_unit(k0)
        # fillers: one list entry per attention slot boundary 1..11
        fillers = (
            [lambda m=m: qk_unit(xq_t, wq_t, QT, m, 2) for m in range(2)]
            + [lambda m=m: qk_unit(xk_t, wk_t, KT, m, 2) for m in range(2)]
            + [lambda k0=k0: v_unit(k0) for k0 in (8, 10)]
            + [lambda m=m: qk_unit(xq_t, wq_t, QT, m, 3) for m in range(2)]
            + [lambda m=m: qk_unit(xk_t, wk_t, KT, m, 3) for m in range(2)]
            + [lambda k0=k0: v_unit(k0) for k0 in (12, 14)]
        )
        inject = {s: [] for s in range(1, 14)}
        slot_for_unit = [1, 2, 3, 4, 5, 6, 7, 8, 9, 10, 12, 13]
        for u, s in zip(fillers, slot_for_unit):
            inject[s].append(u)

        def trim(kt, qc):
            o = kt * P - qc * QC
            return (o, True) if o >= 0 else (0, False)

        def make_pairs(qc):
            kts = list(range(4 * (qc + 1)))
            return [kts[i : i + 2] for i in range(0, len(kts), 2)]

        def score_pair(qc, h, pi, pair):
            """Score matmuls + exp for one kt-pair of (qc, h). Diagonal
            tiles get the triangular mask matmul-preloaded into PSUM; the
            exp span is merged across the pair (the dead gap columns hold
            stale PSUM whose exp lands in at columns PV never reads)."""
            hm, hp = divmod(h, 2)
            hp *= DK
            stt = st_ps.tile([P, 2 * QC], f32, tag="st", name=f"st{qc}_{h}_{pi}")
            for half, kt in enumerate(pair):
                o, diag = trim(kt, qc)
                b = half * QC
                kl = KT[hm][hp : hp + DK, kt * P : (kt + 1) * P]
                if diag:
                    nc.tensor.matmul(
                        stt[:, b + o : b + o + P],
                        lhsT=tri_t,
                        rhs=idn_t,
                        start=True,
                        stop=False,
                        skip_group_check=True,
                    )
                    nc.tensor.matmul(
                        stt[:, b + o : b + o + P],
                        lhsT=kl,
                        rhs=QT[hm][hp : hp + DK, qc * QC + o : qc * QC + o + P],
                        start=False,
                        stop=True,
                        skip_group_check=True,
                    )
                    if o + P < QC:
                        nc.tensor.matmul(
                            stt[:, b + o + P : b + QC],
                            lhsT=kl,
                            rhs=QT[hm][
                                hp : hp + DK, qc * QC + o + P : (qc + 1) * QC
                            ],
                            start=True,
                            stop=True,
                            skip_group_check=True,
                        )
                else:
                    nc.tensor.matmul(
                        stt[:, b : b + QC],
                        lhsT=kl,
                        rhs=QT[hm][hp : hp + DK, qc * QC : (qc + 1) * QC],
                        start=True,
                        stop=True,
                    )
            at = attn_pool.tile(
                [P, 2 * QC], bf16, tag="attn", name=f"a{qc}_{h}_{pi}"
            )
            o0 = trim(pair[0], qc)[0]
            nc.scalar.activation(
                out=at[:, o0 : 2 * QC],
                in_=stt[:, o0 : 2 * QC],
                func=exp_fn,
                scale=0.125,
            )
            return at

        def pv_pair(qc, h, pv, at, pair):
            last = 4 * (qc + 1) - 1
            for half, kt in enumerate(pair):
                o, _ = trim(kt, qc)
                nc.tensor.matmul(
                    pv[:, o:QC],
                    lhsT=vt[kt][:, h * (DK + 1) : (h + 1) * (DK + 1)],
                    rhs=at[:, half * QC + o : (half + 1) * QC],
                    start=(kt == 0),
                    stop=(kt == last),
                    skip_group_check=True,
                )

        def finish_recip(qc, h, pv):
            # 1/sums = exp(-ln(sums)); ln(sums) split hi+lo into two bf16
            # rows so the 1-cycle/col bf16 broadcast keeps fp32 accuracy
            lnf = lnf_pool.tile([1, QC], f32, tag="lnf", name=f"lf{qc}_{h}")
            nc.scalar.activation(out=lnf[:], in_=pv[DK : DK + 1, :], func=ln_fn)
            hi = hilo_pool.tile([1, QC], bf16, tag="lnhi", name=f"lh{qc}_{h}")
            nc.vector.tensor_copy(out=hi[:], in_=lnf[:])
            lo = hilo_pool.tile([1, QC], bf16, tag="lnlo", name=f"ll{qc}_{h}")
            nc.vector.tensor_sub(out=lo[:], in0=lnf[:], in1=hi[:])
            return (hi, lo)

        def emit_bcast_mul(qc, h, pv, inv):
            hi, lo = inv
            hm, hp = divmod(h, 2)
            hp *= DK
            bc = fp_ps.tile([DK, QC], f32, tag="fp", name=f"bc{qc}_{h}")
            nc.tensor.matmul(
                bc[:], lhsT=ones64[:], rhs=hi[:], start=True, stop=False
            )
            nc.tensor.matmul(
                bc[:],
                lhsT=ones64[:],
                rhs=lo[:],
                start=False,
                stop=True,
                skip_group_check=True,
            )
            binv = binv_pool.tile([DK, QC], f32, tag="binv", name=f"bi{qc}_{h}")
            nc.scalar.activation(out=binv[:], in_=bc[:], func=exp_fn, scale=-1.0)
            nc.vector.tensor_mul(
                AT[hm][hp : hp + DK, qc * QC : (qc + 1) * QC],
                pv[0:DK, :],
                binv[:],
            )

        def emit_outproj(qc, split_dma=False):
            # split_dma: DMA each 128-row group as soon as it is cast, on
            # its own queue - the final out-DMA otherwise drains ~16us
            # (1 MiB at the ~55 GB/s per-queue cap) after the last compute
            ob = outp.tile([P, 4, D], bf16, tag="ob", name=f"ob{qc}")
            for g in range(4):
                st = 4 * qc + g
                for nch in range(D // QC):
                    ps = fp_ps.tile([P, QC], f32, tag="fp", name=f"po{st}_{nch}")
                    for m in range(2):
                        nc.tensor.matmul(
                            ps[:],
                            lhsT=AT[m][:, st * P : (st + 1) * P],
                            rhs=wo_t[:, m * D + nch * QC : m * D + (nch + 1) * QC],
                            start=(m == 0),
                            stop=(m == 1),
                        )
                    nc.vector.tensor_copy(
                        out=ob[:, g, nch * QC : (nch + 1) * QC], in_=ps[:]
                    )
                if split_dma:
                    nc.sync.dma_start(
                        out=out_d[st * P : (st + 1) * P, :],
                        in_=ob[:, g, :],
                    )
            if not split_dma:
                nc.sync.dma_start(
                    out=out_d[qc * QC : (qc + 1) * QC, :].rearrange(
                        "(g p) n -> p g n", p=P
                    ),
                    in_=ob[:],
                )

        # flat head-slot pipeline: score-pairs of slot i interleave with
        # PV-pairs of slot i-1; normalize trails by ~1.5 slots; projection
        # filler units and the previous qc's out-projection keep the PE fed
        slots = [(qc, h) for qc in range(nqc) for h in range(HPC)]
        prev = None  # (qc, h, ats)
        pend_bc = []
        for si, (qc, h) in enumerate(slots):
            for u in inject.get(si, []):
                u()
            pairs_now = make_pairs(qc)
            ats_now = []
            pv_prev = None
            if prev is not None:
                pv_prev = pv_ps.tile(
                    [DK + 1, QC], f32, tag="pv", name=f"pv{prev[0]}_{prev[1]}"
                )
            for pi, pair in enumerate(pairs_now):
                ats_now.append((score_pair(qc, h, pi, pair), pair))
                if prev is not None and pi < len(prev[2]):
                    pat, ppair = prev[2][pi]
                    pv_pair(prev[0], prev[1], pv_prev, pat, ppair)
                if pi == 1 and pend_bc:
                    emit_bcast_mul(*pend_bc.pop(0))
                # (qc-1, h3)'s normalize pops at slot (qc, h1) pi==1, so
                # the out-projection of qc-1 is safe only from pi==3 here
                if pi == 3 and h == 1 and qc > 0:
                    emit_outproj(qc - 1)
            if prev is not None:
                inv = finish_recip(prev[0], prev[1], pv_prev)
                pend_bc.append((prev[0], prev[1], pv_prev, inv))
            prev = (qc, h, ats_now)
        # tail: PV + normalize for the last slot, then the last outproj
        pv_last = pv_ps.tile([DK + 1, QC], f32, tag="pv", name="pv_last")
        for pi, (at, pair) in enumerate(prev[2]):
            pv_pair(prev[0], prev[1], pv_last, at, pair)
            if pi == 1 and pend_bc:
                emit_bcast_mul(*pend_bc.pop(0))
        inv = finish_recip(prev[0], prev[1], pv_last)
        pend_bc.append((prev[0], prev[1], pv_last, inv))
        while pend_bc:
            emit_bcast_mul(*pend_bc.pop(0))
        emit_outproj(nqc - 1, split_dma=True)

    return nc


def _build_legacy(mask_mode, seq=S):
    """Fallback for non-causal masks (mask_mode: 'none'|'full')."""
    import concourse.bass as bass
    import concourse.tile as tile
    from concourse import mybir
    from contextlib import ExitStack

    f32 = mybir.dt.float32
    bf16 = mybir.dt.bfloat16
    nqc = seq // QC
    nkt = seq // P
    nd = D // P  # 8 d-chunks

    nc = bass.Bass(num_swdge_queues=4)
    xq_d = nc.dram_tensor("xq_t", [D, seq], bf16, kind="ExternalInput")
    xk_d = nc.dram_tensor("xk_t", [D, seq], bf16, kind="ExternalInput")
    xv_d = nc.dram_tensor("xv_t", [D, seq], bf16, kind="ExternalInput")
    wq_d = nc.dram_tensor("wq_p", [P, D * DKB // P], bf16, kind="ExternalInput")
    wk_d = nc.dram_tensor("wk_p", [P, D * DKB // P], bf16, kind="ExternalInput")
    wv_d = nc.dram_tensor("wv_p", [P, D * DKB // P], bf16, kind="ExternalInput")
    wo_d = nc.dram_tensor("wo_p", [P, DKB * D // P], bf16, kind="ExternalInput")
    if mask_mode == "full":
        maskt_d = nc.dram_tensor("mask_t", [seq, seq], bf16, kind="ExternalInput")
    out_d = nc.dram_tensor("out", [seq, D], f32, kind="ExternalOutput")

    with ExitStack() as ctx:
        tc = ctx.enter_context(tile.TileContext(nc))
        persist = ctx.enter_context(tc.tile_pool(name="persist", bufs=1))

        ones64 = persist.tile([1, DK], f32, tag="ones64")
        nc.vector.memset(ones64[:], 1.0)
        wq_t = persist.tile([P, D * DKB // P], bf16, tag="wq")
        wk_t = persist.tile([P, D * DKB // P], bf16, tag="wk")
        wv_t = persist.tile([P, D * DKB // P], bf16, tag="wv")
        wo_t = persist.tile([P, DKB * D // P], bf16, tag="wo")
        nc.gpsimd.dma_start(out=wq_t[:], in_=wq_d[:, :])
        nc.gpsimd.dma_start(out=wk_t[:], in_=wk_d[:, :])
        nc.gpsimd.dma_start(out=wv_t[:], in_=wv_d[:, :])
        nc.gpsimd.dma_start(out=wo_t[:], in_=wo_d[:, :])

        QT, KT, vt = [], [], []
        for m in range(2):
            QT.append(persist.tile([P, seq], bf16, tag=f"qt{m}", name=f"qt{m}"))
            KT.append(persist.tile([P, seq], bf16, tag=f"kt{m}", name=f"kt{m}"))
        AT = []
        for m in range(2):
            AT.append(persist.tile([P, seq], bf16, tag=f"at{m}", name=f"at{m}"))

        with tc.tile_pool(name="xpool", bufs=1) as xpool, tc.tile_pool(
            name="projp", bufs=2, space="PSUM"
        ) as projp:

            def load_xt(xdram, name):
                t = xpool.tile([P, nd, seq], bf16, tag=name, name=name)
                h = nd // 2
                nc.sync.dma_start(
                    out=t[:, 0:h, :],
                    in_=xdram[: h * P, :].rearrange("(j p) s -> p j s", p=P),
                )
                nc.sync.dma_start(
                    out=t[:, h:nd, :],
                    in_=xdram[h * P :, :].rearrange("(j p) s -> p j s", p=P),
                )
                return t

            xq_t = load_xt(xq_d, "xq")
            xk_t = load_xt(xk_d, "xk")
            xv_t = load_xt(xv_d, "xv")

            def project_T(xt, wtile, res, name):
                ngroups = [
                    list(range(i, min(i + 2, nqc))) for i in range(0, nqc, 2)
                ]
                for m in range(2):
                    for gi, grp in enumerate(ngroups):
                        ps = projp.tile(
                            [P, len(grp) * QC],
                            f32,
                            tag="pj",
                            name=f"ps_{name}{m}_{gi}",
                        )
                        for half, n in enumerate(grp):
                            for j in range(nd):
                                nc.tensor.matmul(
                                    ps[:, half * QC : (half + 1) * QC],
                                    lhsT=wtile[
                                        :, j * DKB + m * P : j * DKB + (m + 1) * P
                                    ],
                                    rhs=xt[:, j, n * QC : (n + 1) * QC],
                                    start=(j == 0),
                                    stop=(j == nd - 1),
                                )
                        nc.vector.tensor_copy(
                            out=res[m][:, grp[0] * QC : (grp[-1] + 1) * QC],
                            in_=ps[:],
                        )

            project_T(xq_t, wq_t, QT, "qt")
            project_T(xk_t, wk_t, KT, "kt")

            for st in range(nkt):
                ps = projp.tile([P, DKB], f32, tag="pj", name=f"ps_v{st}")
                for j in range(nd):
                    nc.tensor.matmul(
                        ps[:],
                        lhsT=xv_t[:, j, st * P : (st + 1) * P],
                        rhs=wv_t[:, j * DKB : (j + 1) * DKB],
                        start=(j == 0),
                        stop=(j == nd - 1),
                    )
                v = persist.tile(
                    [P, HPC * (DK + 1)], bf16, tag=f"v{st}", name=f"v{st}"
                )
                nc.vector.memset(v[:], 1.0)
                nc.vector.tensor_copy(
                    out=v[:].rearrange("p (h w) -> p h w", w=DK + 1)[:, :, 0:DK],
                    in_=ps[:].rearrange("p (h w) -> p h w", w=DK),
                )
                vt.append(v)

        st_ps = ctx.enter_context(tc.tile_pool(name="st_ps", bufs=4, space="PSUM"))
        pv_ps = ctx.enter_context(tc.tile_pool(name="pv_ps", bufs=2, space="PSUM"))
        fp_ps = ctx.enter_context(tc.tile_pool(name="fp_ps", bufs=2, space="PSUM"))
        sc_pool = ctx.enter_context(tc.tile_pool(name="sc_pool", bufs=8))
        attn_pool = ctx.enter_context(tc.tile_pool(name="attn_pool", bufs=8))
        small = ctx.enter_context(tc.tile_pool(name="small", bufs=2))
        outp = ctx.enter_context(tc.tile_pool(name="outp", bufs=2))
        maskp = None
        if mask_mode == "full":
            maskp = ctx.enter_context(tc.tile_pool(name="maskp", bufs=2))

        exp_fn = mybir.ActivationFunctionType.Exp
        ln_fn = mybir.ActivationFunctionType.Ln
        for qc in range(nqc):
            mt = None
            if mask_mode == "full":
                mt = maskp.tile([P, nkt, QC], bf16, tag="mask", name=f"mt{qc}")
                nc.gpsimd.dma_start(
                    out=mt[:],
                    in_=maskt_d[:, qc * QC : (qc + 1) * QC].rearrange(
                        "(kt p) c -> p kt c", p=P
                    ),
                )
            for h in range(HPC):
                hm, hp = divmod(h, 2)
                hp *= DK
                kts = list(range(nkt))
                pairs = [kts[i : i + 2] for i in range(0, len(kts), 2)]

                pv = pv_ps.tile([DK + 1, QC], f32, tag="pv", name=f"pv{qc}_{h}")

                def emit_pv(at, pair, is_last):
                    for half, kt in enumerate(pair):
                        nc.tensor.matmul(
                            pv[:, 0:QC],
                            lhsT=vt[kt][:, h * (DK + 1) : (h + 1) * (DK + 1)],
                            rhs=at[:, half * QC : (half + 1) * QC],
                            start=(kt == 0),
                            stop=(is_last and half == len(pair) - 1),
                            skip_group_check=True,
                        )

                ats = []
                for pi, pair in enumerate(pairs):
                    sc = sc_pool.tile(
                        [P, 2 * QC], f32, tag="sc", name=f"sc{qc}_{h}_{pi}"
                    )
                    for half, kt in enumerate(pair):
                        stt = st_ps.tile(
                            [P, QC], f32, tag="st", name=f"st{qc}_{h}_{kt}"
                        )
                        nc.tensor.matmul(
                            stt[:, 0:QC],
                            lhsT=KT[hm][hp : hp + DK, kt * P : (kt + 1) * P],
                            rhs=QT[hm][hp : hp + DK, qc * QC : (qc + 1) * QC],
                            start=True,
                            stop=True,
                        )
                        dst = sc[:, half * QC : (half + 1) * QC]
                        if mask_mode == "full":
                            nc.vector.tensor_add(
                                out=dst, in0=stt[:, 0:QC], in1=mt[:, kt, :]
                            )
                        else:
                            nc.vector.tensor_copy(out=dst, in_=stt[:, 0:QC])
                    at = attn_pool.tile(
                        [P, 2 * QC], bf16, tag="attn", name=f"a{qc}_{h}_{pi}"
                    )
                    nc.scalar.activation(
                        out=at[:], in_=sc[:], func=exp_fn, scale=0.125
                    )
                    ats.append((at, pair))
                for at, pair in ats:
                    emit_pv(at, pair, pair is pairs[-1])
                lns = small.tile([1, QC], f32, tag="lns", name=f"ln{qc}_{h}")
                nc.scalar.activation(
                    out=lns[:], in_=pv[DK : DK + 1, :], func=ln_fn
                )
                bcp = fp_ps.tile([DK, QC], f32, tag="fp", name=f"bcp{qc}_{h}")
                nc.tensor.matmul(
                    bcp[:], lhsT=ones64[:], rhs=lns[:], start=True, stop=True
                )
                bc = small.tile([DK, QC], f32, tag="bcast", name=f"bc{qc}_{h}")
                nc.scalar.activation(
                    out=bc[:], in_=bcp[:], func=exp_fn, scale=-1.0
                )
                nc.vector.tensor_mul(
                    AT[hm][hp : hp + DK, qc * QC : (qc + 1) * QC],
                    pv[0:DK, :],
                    bc[:],
                )

            for j2 in range(qc * (QC // (2 * P)), (qc + 1) * (QC // (2 * P))):
                ob = outp.tile([P, 2, D], f32, tag="ob", name=f"ob{j2}")
                for g in range(2):
                    st = 2 * j2 + g
                    for nch in range(D // QC):
                        ps = fp_ps.tile(
                            [P, QC], f32, tag="fp", name=f"ps_o{st}_{nch}"
                        )
                        for m in range(2):
                            nc.tensor.matmul(
                                ps[:],
                                lhsT=AT[m][:, st * P : (st + 1) * P],
                                rhs=wo_t[
                                    :, m * D + nch * QC : m * D + (nch + 1) * QC
                                ],
                                start=(m == 0),
                                stop=(m == 1),
                            )
                        nc.vector.tensor_copy(
                            out=ob[:, g, nch * QC : (nch + 1) * QC], in_=ps[:]
                        )
                nc.sync.dma_start(
                    out=out_d[j2 * 2 * P : (j2 + 1) * 2 * P, :].rearrange(
                        "(g p) n -> p g n", p=P
                    ),
                    in_=ob[:],
                )

    return nc


def _split_multi_waits(nc):
    """This toolchain's walrus accepts at most one sync-wait per
    instruction. Hoist extra waits onto preceding same-engine NoOps —
    engine streams execute in order, so a NoOp that blocks on a
    semaphore gates everything after it (including HWDGE descriptor
    enqueues, which happen when the issuing engine's sequencer reaches
    the DMA instruction)."""
    import bass_rust

    ctr = 0
    for f in nc.m.functions:
        for bb in f.blocks:
            insts = bb.instructions
            new = []
            changed = False
            for inst in insts:
                si = inst.sync_info
                if si is not None and len(si.on_wait) > 1:
                    waits = list(si.on_wait)
                    for w in waits[:-1]:
                        ctr += 1
                        nop = bass_rust.InstNoOp(
                            name=f"wsplit_{ctr}", ins=[], outs=[]
                        )
                        nop.engine = inst.engine
                        nop.sync_info = bass_rust.SyncInfo(
                            on_wait=[w], on_update=[]
                        )
                        new.append(nop)
                    inst.sync_info = bass_rust.SyncInfo(
                        on_wait=[waits[-1]], on_update=list(si.on_update)
                    )
                    changed = True
                new.append(inst)
            if changed:
                try:
                    bb.instructions = new
                except AttributeError:
                    insts.clear()
                    insts.extend(new)
    return nc


def _get_nc(mask_mode, seq=S, split_waits=True):
    key = (mask_mode, seq, split_waits)
    if key not in _nc_cache:
        if mask_mode == "causal":
            nc = _build_causal(seq)
        else:
            nc = _build_legacy(mask_mode, seq)
        if split_waits:
            _split_multi_waits(nc)
        _nc_cache[key] = nc
    return _nc_cache[key]


def _pack_w(w_slice_T, ncols):
    # [D_in, ncols] -> [128, D_in/128 * ncols]: col block j holds rows j*128..
    d_in = w_slice_T.shape[0]
    return (
        w_slice_T.reshape(d_in // P, P, ncols).transpose(1, 0, 2).reshape(P, -1)
    )


def _tri_np():
    # preload = tri.T @ I : psum[p, j] = tri[j, p] = MASKVAL where j < p
    j = np.arange(P)[:, None]
    p = np.arange(P)[None, :]
    return np.where(j < p, np.float32(MASKVAL), np.float32(0.0)).astype(BF16)


def _detect_mask_mode(mask):
    if not mask.any():
        return "none"
    causal = np.triu(np.ones((mask.shape[1], mask.shape[2]), bool), k=1)
    if all(np.array_equal(mask[b], causal) for b in range(mask.shape[0])):
        return "causal"
    return "full"


def _make_in_maps(query, key, value, mask, w_q, w_k, w_v, w_o, mask_mode, seq=S):
    per_batch = []
    for b in range(B):
        d = {
            "xq_t": np.ascontiguousarray(query[b].T).astype(BF16),
            "xk_t": np.ascontiguousarray(key[b].T).astype(BF16),
            "xv_t": np.ascontiguousarray(value[b].T).astype(BF16),
        }
        if mask_mode == "full":
            d["mask_t"] = np.where(
                mask[b].T, np.float32(NEG), np.float32(0.0)
            ).astype(BF16)
        per_batch.append(d)
    per_hg = []
    for hg in range(HGROUPS):
        rows = slice(hg * DKB, (hg + 1) * DKB)
        wq_p = _pack_w(w_q[rows, :].T.astype(BF16), DKB)
        wk_p = _pack_w(w_k[rows, :].T.astype(BF16), DKB)
        wv_p = _pack_w(w_v[rows, :].T.astype(BF16), DKB)
        wo_p = _pack_w(w_o[:, rows].T.astype(BF16), D)
        if mask_mode == "causal":
            wrest = np.concatenate(
                [wo_p, _tri_np(), np.eye(P, dtype=BF16)], axis=1
            )
            per_hg.append(
                {
                    "wq_p": wq_p,
                    "wk_p": wk_p,
                    "wv_p": wv_p,
                    "wrest": np.ascontiguousarray(wrest),
                }
            )
        else:
            per_hg.append(
                {"wq_p": wq_p, "wk_p": wk_p, "wv_p": wv_p, "wo_p": wo_p}
            )
    in_maps = []
    for c in range(NCORE):
        b, hg = divmod(c, HGROUPS)
        im = dict(per_batch[b])
        im.update(per_hg[hg])
        in_maps.append(im)
    return in_maps


def _run(inputs, trace=False):
    from concourse.bass_utils import run_bass_kernel_spmd

    query = np.asarray(inputs["query"], np.float32)
    key = np.asarray(inputs["key"], np.float32)
    value = np.asarray(inputs["value"], np.float32)
    mask = np.asarray(inputs["mask"], bool)
    w_q = np.asarray(inputs["w_q"], np.float32)
    w_k = np.asarray(inputs["w_k"], np.float32)
    w_v = np.asarray(inputs["w_v"], np.float32)
    w_o = np.asarray(inputs["w_o"], np.float32)
    b_o = np.asarray(inputs["b_o"], np.float32)
    assert query.shape == (B, S, D), query.shape

    mask_mode = _detect_mask_mode(mask)
    nc = _get_nc(mask_mode)
    in_maps = _make_in_maps(query, key, value, mask, w_q, w_k, w_v, w_o, mask_mode)
    res = run_bass_kernel_spmd(nc, in_maps, list(range(NCORE)), trace=trace)
    outs = [np.asarray(r["out"], np.float32) for r in res.results]
    full = np.empty((B, S, D), np.float32)
    for b in range(B):
        full[b] = outs[HGROUPS * b]
        for i in range(1, HGROUPS):
            full[b] += outs[HGROUPS * b + i]
    full += b_o[None, None, :]
    return full, res


def kernel(**inputs):
    out, _ = _run(inputs, trace=False)
    return out


if __name__ == "__main__":
    import tempfile
    from concourse.bass_utils import compile_bass_kernel

    mode = sys.argv[1] if len(sys.argv) > 1 else "causal"
    nc = _get_nc(mode)
    from collections import Counter

    c = Counter()
    for name, inst in nc.inst_map.items():
        if "DMACopy" in type(inst).__name__:
            c[str(inst).count("wait:")] += 1
    print("DMA wait dist:", dict(c))
    td = tempfile.mkdtemp()
    p = compile_bass_kernel(nc, td)
    print("COMPILED OK:", p)
